# revision 1
# baseline (speedup 1.0000x reference)
"""DualAN (normalization) Trainium2 Bass kernel.

kernel(**inputs) takes FULL inputs (batch_x [32,720,862] f32 + MLP weights),
shards batch across 8 NeuronCores (pure data parallel), runs one Bass program
per core on its [4,720,862] slice, returns FULL [32,720,3448] f32.

Per-core pipeline, per (batch, 431-channel half), time-major [t,e] layouts:
  1. split-fp16 DFT (3 matmuls/chunk, fp32-class accuracy) -> Xr,Xi [f,e] f32
  2. mag2 = Xr^2+Xi^2; PE-transpose -> [e,f]; top-20 threshold per channel via
     vector.max + match_replace x3 (rank 20 = 4th value of 3rd top-8 round)
  3. mask = mag2 >= v20; PE-transpose mask back -> [f,e]; masked coefs (fp16)
  4. iDFT (fp16) -> x_filt; ni = x - x_filt
  5. sliding-window mean/std via band matmuls (window 24, edge replication
     folded into the band matrix); norm = (ni - mean)/sqrt(var + 1e-5)
  6. three MLPs (fp16, feature-major: weights as lhsT, activations as rhs),
     bias+ReLU fused into ACT-engine PSUM evacuations.
"""

import numpy as np
from contextlib import ExitStack

B, S, E = 32, 720, 862
F = 361          # rfft bins
FP = 362         # padded even
PRED = 720
WIN = 24
EPS = 1e-5
NCORES = 8
BL = B // NCORES

TC = 120         # time chunk
NT = 6
FCH = [(0, 121), (121, 121), (242, 120)]       # f-chunks (sum 362)
EH = [(0, 431), (431, 431)]                    # e halves
EW = 431
ECH = [(0, 128), (128, 128), (256, 128), (384, 47)]  # e chunks within a half

_cache = {}
_MARKS = []


def _f16(a):
    return np.asarray(a).astype(np.float16)


def _f16split(a):
    hi = a.astype(np.float16)
    lo = (a.astype(np.float32) - hi.astype(np.float32)).astype(np.float16)
    return hi, lo


def _host_constants():
    t = np.arange(S, dtype=np.float64)
    f = np.arange(F, dtype=np.float64)
    ang = 2.0 * np.pi * np.outer(t, f) / S          # [S, F]

    # folded DFT: Xr = sum_{t<361} u[t] cos(2pi f t/S), u = x[t]+x[S-t] (t in
    # [1,360)), u[0]=x[0], u[360]=x[360]; Xi = sum v[t] (-sin), v = x[t]-x[S-t].
    tf = np.arange(363, dtype=np.float64)
    angf = 2.0 * np.pi * np.outer(tf, f) / S        # [363, F]
    Cf = np.cos(angf)
    Sf = -np.sin(angf)
    Cf[361:] = 0.0
    Sf[361:] = 0.0
    Sf[0] = 0.0
    Sf[360] = 0.0
    Cf = np.concatenate([Cf.astype(np.float32), np.zeros((363, 1), np.float32)], 1)
    Sf = np.concatenate([Sf.astype(np.float32), np.zeros((363, 1), np.float32)], 1)
    chh, chl = _f16split(Cf)
    shh, shl = _f16split(Sf)

    # J permutation blocks: xrev[oc*121+m] = x[S - t] for t = oc*121+m in [1,360)
    jrev = np.zeros((4, TC, 121), np.float32)
    for tt in range(1, 360):
        r = S - tt
        csrc, k = divmod(r, TC)
        oc, m = divmod(tt, 121)
        idx = {(0, 5): 0, (1, 4): 1, (1, 3): 2, (2, 3): 3}[(oc, csrc)]
        jrev[idx, k, m] = 1.0

    w = np.full(F, 2.0); w[0] = 1.0; w[360] = 1.0
    c2 = (w[:, None] * np.cos(ang.T) / S)
    s2 = (w[:, None] * (-np.sin(ang.T)) / S)
    c2 = np.concatenate([c2, np.zeros((1, S))], 0).astype(np.float32)
    s2 = np.concatenate([s2, np.zeros((1, S))], 0).astype(np.float32)

    # band chunk j main slab = rows [_band_src(j), +128); neighbor slab
    # _band_nb(j) catches the window rows that fall outside the main slab.
    bands = np.zeros((NT, 2, 128, TC), np.float64)
    for j in range(NT):
        src_m = _band_src(j)
        src_n = _band_src(_band_nb(j))
        for tt in range(TC):
            g = TC * j + tt
            for q in range(g - WIN // 2, g + WIN // 2):
                qq = min(max(q, 0), S - 1)
                rm = qq - src_m
                if 0 <= rm < 128:
                    bands[j, 0, rm, tt] += 1.0
                else:
                    rn = qq - src_n
                    assert 0 <= rn < 128, (j, tt, qq)
                    bands[j, 1, rn, tt] += 1.0
    bands = bands.astype(np.float32)
    ident = np.eye(128, dtype=np.float32)
    return dict(
        chh=chh, chl=chl, shh=shh, shl=shl, jrev=_f16(jrev),
        c2=_f16(c2), s2=_f16(s2), band=_f16(bands),
        idf=ident, idh=_f16(ident),
    )


def _band_src(j):
    if j == 0:
        return 0
    if j == NT - 1:
        return S - 128
    return TC * j - 12


def _band_nb(j):
    return 4 if j == NT - 1 else j + 1


def _build_program():
    import concourse.tile as tile
    from concourse import bacc, mybir

    dt = mybir.dt
    AF = mybir.ActivationFunctionType
    OP = mybir.AluOpType

    nc = bacc.Bacc("TRN2", target_bir_lowering=False, debug=False)

    x_d = nc.dram_tensor("x", [BL, S, E], dt.float32, kind="ExternalInput")
    chh_d = nc.dram_tensor("chh", [363, FP], dt.float16, kind="ExternalInput")
    chl_d = nc.dram_tensor("chl", [363, FP], dt.float16, kind="ExternalInput")
    shh_d = nc.dram_tensor("shh", [363, FP], dt.float16, kind="ExternalInput")
    shl_d = nc.dram_tensor("shl", [363, FP], dt.float16, kind="ExternalInput")
    c2_d = nc.dram_tensor("c2", [FP, S], dt.float16, kind="ExternalInput")
    s2_d = nc.dram_tensor("s2", [FP, S], dt.float16, kind="ExternalInput")
    band_d = nc.dram_tensor("band", [NT, 2, 128, TC], dt.float16, kind="ExternalInput")
    jrev_d = nc.dram_tensor("jrev", [4, TC, 121], dt.float16, kind="ExternalInput")
    idf_d = nc.dram_tensor("idf", [128, 128], dt.float32, kind="ExternalInput")
    idh_d = nc.dram_tensor("idh", [128, 128], dt.float16, kind="ExternalInput")
    wf1_d = nc.dram_tensor("wf1", [S, 64], dt.float16, kind="ExternalInput")
    wf2_d = nc.dram_tensor("wf2", [64 + S, 128], dt.float16, kind="ExternalInput")
    wf3_d = nc.dram_tensor("wf3", [128, PRED], dt.float16, kind="ExternalInput")
    wp1_d = nc.dram_tensor("wp1", [S, 256], dt.float16, kind="ExternalInput")
    wp2_d = nc.dram_tensor("wp2", [256 + S, 512], dt.float16, kind="ExternalInput")
    wp3_d = nc.dram_tensor("wp3", [512, PRED], dt.float16, kind="ExternalInput")
    bf1_d = nc.dram_tensor("bf1", [64], dt.float32, kind="ExternalInput")
    bf2_d = nc.dram_tensor("bf2", [128], dt.float32, kind="ExternalInput")
    bf3_d = nc.dram_tensor("bf3", [PRED], dt.float32, kind="ExternalInput")
    bp1_d = nc.dram_tensor("bp1", [256], dt.float32, kind="ExternalInput")
    bp2_d = nc.dram_tensor("bp2", [512], dt.float32, kind="ExternalInput")
    bp3_d = nc.dram_tensor("bp3", [PRED], dt.float32, kind="ExternalInput")
    out_d = nc.dram_tensor("out", [BL, S, 4 * E], dt.float32, kind="ExternalOutput")

    with tile.TileContext(nc) as tc, ExitStack() as ctx:
        const = ctx.enter_context(tc.tile_pool(name="const", bufs=1))
        big = ctx.enter_context(tc.tile_pool(name="big", bufs=1))
        med = ctx.enter_context(tc.tile_pool(name="med", bufs=1))
        tmp = ctx.enter_context(tc.tile_pool(name="tmp", bufs=2))
        ps = ctx.enter_context(tc.tile_pool(name="ps", bufs=8, space="PSUM"))

        # ---------------- constants ----------------
        def load3(d, width, dtype):
            t_ = const.tile([121, 3, width], dtype, name=d.name + "_t")
            nc.sync.dma_start(t_[:], d.ap().rearrange("(c p) f -> p c f", p=121))
            return t_

        jrev_t = const.tile([TC, 4, 121], dt.float16)
        nc.sync.dma_start(jrev_t[:], jrev_d.ap().rearrange("c p m -> p c m"))

        chh_t = load3(chh_d, FP, dt.float16)
        chl_t = load3(chl_d, FP, dt.float16)
        shh_t = load3(shh_d, FP, dt.float16)
        shl_t = load3(shl_d, FP, dt.float16)

        idf_t = const.tile([128, 128], dt.float32)
        nc.sync.dma_start(idf_t[:], idf_d.ap()[:])
        eps_t = const.tile([128, 1], dt.float32)
        nc.vector.memset(eps_t[:], EPS)

        c2_t, s2_t = [], []
        idh_l, band_l = [], []

        def load_consts2():
            for ci, (f0, fw) in enumerate(FCH):
                a = const.tile([fw, S], dt.float16, name=f"c2_{ci}")
                nc.sync.dma_start(a[:], c2_d.ap()[f0:f0 + fw, :])
                c2_t.append(a)
                bb = const.tile([fw, S], dt.float16, name=f"s2_{ci}")
                nc.sync.dma_start(bb[:], s2_d.ap()[f0:f0 + fw, :])
                s2_t.append(bb)
            bt = const.tile([128, NT, 2, TC], dt.float16, name="band_t")
            nc.sync.dma_start(bt[:], band_d.ap().rearrange("c n p f -> p c n f"))
            band_l.append(bt)
            ih = const.tile([128, 128], dt.float16, name="idh_t")
            nc.sync.dma_start(ih[:], idh_d.ap()[:])
            idh_l.append(ih)

        W = {}
        _wstate = {}

        def load_weights():
            if _wstate.get('done'):
                return
            _wstate['done'] = True
            W['wf1_t'] = const.tile([TC, NT, 64], dt.float16, name='wf1_t')
            nc.sync.dma_start(W['wf1_t'][:], wf1_d.ap().rearrange("(c p) m -> p c m", p=TC))
            W['wf2h_t'] = const.tile([64, 128], dt.float16, name='wf2h_t')
            nc.sync.dma_start(W['wf2h_t'][:], wf2_d.ap()[0:64, :])
            W['wf2x_t'] = const.tile([TC, NT, 128], dt.float16, name='wf2x_t')
            nc.sync.dma_start(W['wf2x_t'][:], wf2_d.ap()[64:, :].rearrange("(c p) m -> p c m", p=TC))
            W['wf3_t'] = const.tile([128, NT, TC], dt.float16, name='wf3_t')
            nc.sync.dma_start(W['wf3_t'][:], wf3_d.ap().rearrange("k (c m) -> k c m", m=TC))
            W['wp1_t'] = const.tile([TC, NT, 256], dt.float16, name='wp1_t')
            nc.sync.dma_start(W['wp1_t'][:], wp1_d.ap().rearrange("(c p) m -> p c m", p=TC))
            W['wp2h_t'] = const.tile([128, 2, 512], dt.float16, name='wp2h_t')
            nc.sync.dma_start(W['wp2h_t'][:], wp2_d.ap()[0:256, :].rearrange("(c p) m -> p c m", p=128))
            W['wp2x_t'] = const.tile([TC, NT, 512], dt.float16, name='wp2x_t')
            nc.sync.dma_start(W['wp2x_t'][:], wp2_d.ap()[256:, :].rearrange("(c p) m -> p c m", p=TC))
            W['wp3_t'] = const.tile([128, 4, NT, TC], dt.float16, name='wp3_t')
            for kc in range(4):
                nc.sync.dma_start(
                    W['wp3_t'][:, kc, :, :],
                    wp3_d.ap()[128 * kc:128 * (kc + 1), :].rearrange("k (c m) -> k c m", m=TC))

            W['bf1_t'] = const.tile([64, 1], dt.float32, name='bf1_t')
            nc.sync.dma_start(W['bf1_t'][:], bf1_d.ap().rearrange("(p o) -> p o", o=1))
            W['bf2_t'] = const.tile([128, 1], dt.float32, name='bf2_t')
            nc.sync.dma_start(W['bf2_t'][:], bf2_d.ap().rearrange("(p o) -> p o", o=1))
            W['bf3_t'] = const.tile([TC, NT], dt.float32, name='bf3_t')
            nc.sync.dma_start(W['bf3_t'][:], bf3_d.ap().rearrange("(c p) -> p c", p=TC))
            W['bp1_t'] = const.tile([128, 2], dt.float32, name='bp1_t')
            nc.sync.dma_start(W['bp1_t'][:], bp1_d.ap().rearrange("(c p) -> p c", p=128))
            W['bp2_t'] = const.tile([128, 4], dt.float32, name='bp2_t')
            nc.sync.dma_start(W['bp2_t'][:], bp2_d.ap().rearrange("(c p) -> p c", p=128))
            W['bp3_t'] = const.tile([TC, NT], dt.float32, name='bp3_t')
            nc.sync.dma_start(W['bp3_t'][:], bp3_d.ap().rearrange("(c p) -> p c", p=TC))

        cosm = (chh_t, chl_t)
        sinm = (shh_t, shl_t)

        def mark(label):
            _MARKS.append((label, nc.next_id()))

        # -------------- per (batch, e-half) pipeline, 2-stage software pipeline --
        def stage_a(b, e0):
                mark(f"A:load b{b} e{e0}")
                # load + fp16 split of x half
                xh = big.tile([TC, NT, EW], dt.float16, tag="xh", bufs=2)
                xl = big.tile([TC, NT, EW], dt.float16, tag="xl", bufs=2)
                for k in range(NT):
                    xtc = tmp.tile([TC, EW], dt.float32, tag="xtc", bufs=4)
                    nc.sync.dma_start(
                        xtc[:], x_d.ap()[b, TC * k:TC * (k + 1), e0:e0 + EW])
                    nc.gpsimd.tensor_copy(xh[:, k, :], xtc[:])
                    nc.vector.tensor_tensor(xl[:, k, :], xtc[:], xh[:, k, :], OP.subtract)

                mark("A:dft")
                # ---- fold: u = x + x_rev, v = x - x_rev (121-chunked) ----
                JMAP = [[(0, 5)], [(1, 4), (2, 3)], [(3, 3)]]
                uh = [big.tile([121, EW], dt.float16, tag=f"uh{o}", name=f"uh{o}")
                      for o in range(3)]
                ul = [big.tile([121, EW], dt.float16, tag=f"ul{o}", name=f"ul{o}")
                      for o in range(3)]
                vh = [big.tile([121, EW], dt.float16, tag=f"vh{o}", name=f"vh{o}")
                      for o in range(3)]
                vl = [big.tile([121, EW], dt.float16, tag=f"vl{o}", name=f"vl{o}")
                      for o in range(3)]
                for oc in range(3):
                    pr = ps.tile([128, EW], dt.float32, tag="ps")
                    pairs = JMAP[oc]
                    for pi_, (jidx, csrc) in enumerate(pairs):
                        nc.tensor.matmul(pr[:121, :], jrev_t[:, jidx, :],
                                         xh[:, csrc, :], start=(pi_ == 0), stop=False)
                        nc.tensor.matmul(pr[:121, :], jrev_t[:, jidx, :],
                                         xl[:, csrc, :], start=False,
                                         stop=(pi_ == len(pairs) - 1))
                    xfw = tmp.tile([121, EW], dt.float32, tag="xtc", bufs=4)
                    nc.sync.dma_start(
                        xfw[:], x_d.ap()[b, 121 * oc:121 * oc + 121, e0:e0 + EW])
                    u32 = tmp.tile([121, EW], dt.float32, tag="sq2")
                    nc.vector.scalar_tensor_tensor(u32[:], pr[:121, :], 1.0,
                                                   xfw[:], OP.mult, OP.add)
                    v32 = tmp.tile([121, EW], dt.float32, tag="msq")
                    nc.vector.scalar_tensor_tensor(v32[:], pr[:121, :], -1.0,
                                                   xfw[:], OP.mult, OP.add)
                    nc.gpsimd.tensor_copy(uh[oc][:], u32[:])
                    nc.vector.tensor_tensor(ul[oc][:], u32[:], uh[oc][:], OP.subtract)
                    nc.gpsimd.tensor_copy(vh[oc][:], v32[:])
                    nc.vector.tensor_tensor(vl[oc][:], v32[:], vh[oc][:], OP.subtract)

                return dict(b=b, e0=e0, xh=xh, uh=uh, ul=ul, vh=vh, vl=vl)

        def stage_a2(st):
                b, e0, xh = st["b"], st["e0"], st["xh"]
                uh, ul, vh, vl = st["uh"], st["ul"], st["vh"], st["vl"]
                # ---- folded DFT -> Xr/Xi [f, e] f32; mag2 ----
                xr_sb, xi_sb, mag2 = [], [], []
                for ci, (f0, fw) in enumerate(FCH):
                    xr_sb.append(big.tile([fw, EW], dt.float32, tag=f"xr{ci}", name=f"xr{ci}", bufs=2))
                    xi_sb.append(big.tile([fw, EW], dt.float32, tag=f"xi{ci}", name=f"xi{ci}", bufs=2))
                    mag2.append(big.tile([fw, EW], dt.float32, tag=f"mag2{ci}", name=f"mag2{ci}"))
                for mats, src_hl, dst in ((cosm, (uh, ul), xr_sb),
                                          (sinm, (vh, vl), xi_sb)):
                    sh_, sl_ = src_hl
                    for ci, (f0, fw) in enumerate(FCH):
                        p = ps.tile([128, EW], dt.float32, tag="ps")
                        for k in range(3):
                            nc.tensor.matmul(p[:fw, :], mats[0][:, k, f0:f0 + fw],
                                             sh_[k][:], start=(k == 0), stop=False)
                            nc.tensor.matmul(p[:fw, :], mats[1][:, k, f0:f0 + fw],
                                             sh_[k][:], start=False, stop=False)
                            nc.tensor.matmul(p[:fw, :], mats[0][:, k, f0:f0 + fw],
                                             sl_[k][:], start=False, stop=(k == 2))
                        nc.scalar.copy(dst[ci][:], p[:fw, :])
                for ci, (f0, fw) in enumerate(FCH):
                    nc.scalar.square(mag2[ci][:], xi_sb[ci][:])
                    sq2 = tmp.tile([128, EW], dt.float32, tag="sq2")
                    nc.scalar.square(sq2[:fw, :], xr_sb[ci][:])
                    nc.vector.tensor_tensor(mag2[ci][:], mag2[ci][:], sq2[:fw, :], OP.add)

                mark("A:mag2T")
                # ---- transpose mag2 -> e-major ----
                mag2T = big.tile([128, len(ECH), FP], dt.float32, tag="mag2T")
                for ci, (f0, fw) in enumerate(FCH):
                    for ei, (ee0, ew) in enumerate(ECH):
                        pt = ps.tile([128, EW], dt.float32, tag="ps")
                        nc.tensor.transpose(pt[:ew, :fw], mag2[ci][:, ee0:ee0 + ew],
                                            idf_t[:fw, :fw])
                        nc.vector.tensor_copy(mag2T[:ew, ei, f0:f0 + fw], pt[:ew, :fw])

                mark("A:sel")
                # ---- top-20 threshold + mask ----
                mask = big.tile([128, len(ECH), FP], dt.float16, tag="mask", bufs=2)
                for ei, (ee0, ew) in enumerate(ECH):
                    m1 = tmp.tile([128, 8], dt.float32, tag="m1")
                    nc.vector.max(m1[:ew, :], mag2T[:ew, ei, :])
                    r1 = tmp.tile([128, FP], dt.float32, tag="r1")
                    nc.vector.match_replace(r1[:ew, :], m1[:ew, :], mag2T[:ew, ei, :], -1e30)
                    m2 = tmp.tile([128, 8], dt.float32, tag="m2")
                    nc.vector.max(m2[:ew, :], r1[:ew, :])
                    r2 = tmp.tile([128, FP], dt.float32, tag="r2")
                    nc.vector.match_replace(r2[:ew, :], m2[:ew, :], r1[:ew, :], -1e30)
                    m3 = tmp.tile([128, 8], dt.float32, tag="m3")
                    nc.vector.max(m3[:ew, :], r2[:ew, :])
                    nc.gpsimd.tensor_scalar(mask[:ew, ei, :], mag2T[:ew, ei, :],
                                            m3[:ew, 3:4], None, OP.is_ge)
                st.update(xr_sb=xr_sb, xi_sb=xi_sb, mask=mask)
                return st

        def stage_b1(st):
                b, e0, xh = st["b"], st["e0"], st["xh"]
                xr_sb, xi_sb, mask = st["xr_sb"], st["xi_sb"], st["mask"]
                mark("B:maskT")
                # ---- transpose mask -> f-major; masked coefs ----
                xrm = [big.tile([fw, EW], dt.float16, tag=f"xrm{ci}", name=f"xrm{ci}")
                       for ci, (f0, fw) in enumerate(FCH)]
                xim = [big.tile([fw, EW], dt.float16, tag=f"xim{ci}", name=f"xim{ci}")
                       for ci, (f0, fw) in enumerate(FCH)]
                for ci, (f0, fw) in enumerate(FCH):
                    mTc = tmp.tile([128, EW], dt.float16, tag="mTc")
                    for ei, (ee0, ew) in enumerate(ECH):
                        pt = ps.tile([128, EW], dt.float16, tag="ps")
                        nc.tensor.transpose(pt[:fw, :ew], mask[:ew, ei, f0:f0 + fw],
                                            idh_l[0][:ew, :ew])
                        nc.vector.tensor_copy(mTc[:fw, ee0:ee0 + ew], pt[:fw, :ew])
                    nc.vector.tensor_tensor(xrm[ci][:], xr_sb[ci][:], mTc[:fw, :], OP.mult)
                    nc.gpsimd.tensor_tensor(xim[ci][:], xi_sb[ci][:], mTc[:fw, :], OP.mult)

                mark("B:idft")
                # ---- iDFT -> x_filt; ni ----
                xfb = big.tile([TC, NT, EW], dt.float16, tag="xfb")
                nib = big.tile([TC, NT, EW], dt.float16, tag="nib")
                for j in range(NT):
                    t0 = TC * j
                    p = ps.tile([128, EW], dt.float32, tag="ps")
                    for ci in range(len(FCH)):
                        nc.tensor.matmul(p[:TC, :], c2_t[ci][:, t0:t0 + TC],
                                         xrm[ci][:], start=(ci == 0), stop=False)
                        nc.tensor.matmul(p[:TC, :], s2_t[ci][:, t0:t0 + TC],
                                         xim[ci][:], start=False,
                                         stop=(ci == len(FCH) - 1))
                    nc.scalar.copy(xfb[:, j, :], p[:TC, :])
                    nc.vector.scalar_tensor_tensor(nib[:, j, :], p[:TC, :], -1.0,
                                                   xh[:, j, :], OP.mult, OP.add)

                mark("B:band")
                # ---- band layout + squares ----
                nibnd = [big.tile([128, EW], dt.float16, tag=f"nibnd{j}",
                                  name=f"nibnd{j}") for j in range(NT)]
                sqbnd = [big.tile([128, EW], dt.float16, tag=f"sqbnd{j}",
                                  name=f"sqbnd{j}") for j in range(NT)]
                for j in range(NT):
                    g0 = _band_src(j)
                    c0, p0 = divmod(g0, TC)
                    n0 = min(TC - p0, 128)
                    nc.sync.dma_start(nibnd[j][0:n0, :], nib[p0:p0 + n0, c0, :])
                    left = 128 - n0
                    while left > 0:
                        c0 += 1
                        n1 = min(TC, left)
                        nc.sync.dma_start(nibnd[j][128 - left:128 - left + n1, :],
                                          nib[0:n1, c0, :])
                        left -= n1
                    if j % 2 == 0:
                        nc.scalar.square(sqbnd[j][:], nibnd[j][:])
                    else:
                        nc.vector.tensor_tensor(sqbnd[j][:], nibnd[j][:],
                                                nibnd[j][:], OP.mult)
                st["xfb"], st["nib"] = xfb, nib
                st["nibnd"], st["sqbnd"] = nibnd, sqbnd

        def stage_b2(st):
                b, e0, xh = st["b"], st["e0"], st["xh"]
                xfb, nib = st["xfb"], st["nib"]
                nibnd, sqbnd = st["nibnd"], st["sqbnd"]
                
                mark("B:mlpf")
                # ---- MLP freq ----
                h1f = med.tile([64, EW], dt.float16, tag="h1f")
                p = ps.tile([128, EW], dt.float32, tag="ps")
                for k in range(NT):
                    nc.tensor.matmul(p[:64, :], W['wf1_t'][:, k, :], xfb[:, k, :],
                                     start=(k == 0), stop=(k == NT - 1))
                nc.scalar.activation(h1f[:], p[:64, :], AF.Relu, bias=W['bf1_t'][:, 0:1])
                h2f = med.tile([128, EW], dt.float16, tag="h2f")
                p = ps.tile([128, EW], dt.float32, tag="ps")
                for k in range(NT):
                    nc.tensor.matmul(p[:], W['wf2x_t'][:, k, :], xh[:, k, :],
                                     start=(k == 0), stop=False)
                nc.tensor.matmul(p[:], W['wf2h_t'][:], h1f[:], start=False, stop=True)
                nc.scalar.activation(h2f[:], p[:], AF.Relu, bias=W['bf2_t'][:, 0:1])
                for j in range(NT):
                    p = ps.tile([128, EW], dt.float32, tag="ps")
                    nc.tensor.matmul(p[:TC, :], W['wf3_t'][:, j, :], h2f[:],
                                     start=True, stop=True)
                    o = tmp.tile([TC, EW], dt.float32, tag="of", bufs=2)
                    nc.scalar.activation(o[:], p[:TC, :], AF.Identity,
                                         bias=W['bf3_t'][:, j:j + 1])
                    nc.sync.dma_start(
                        out_d.ap()[b, TC * j:TC * (j + 1), E + e0:E + e0 + EW], o[:])

                mark("B:stats")
                # ---- window stats + norm ----
                meanb = [big.tile([TC, EW], dt.float16, tag=f"meanb{j}",
                                  name=f"meanb{j}") for j in range(NT)]
                stdb = [big.tile([TC, EW], dt.float16, tag=f"stdb{j}",
                                 name=f"stdb{j}") for j in range(NT)]
                for j in range(NT):
                    nb = _band_nb(j)
                    p1 = ps.tile([128, EW], dt.float32, tag="ps")
                    nc.tensor.matmul(p1[:TC, :], band_l[0][:, j, 0, :], nibnd[j][:],
                                     start=True, stop=False)
                    nc.tensor.matmul(p1[:TC, :], band_l[0][:, j, 1, :], nibnd[nb][:],
                                     start=False, stop=True)
                    p2 = ps.tile([128, EW], dt.float32, tag="ps")
                    nc.tensor.matmul(p2[:TC, :], band_l[0][:, j, 0, :], sqbnd[j][:],
                                     start=True, stop=False)
                    nc.tensor.matmul(p2[:TC, :], band_l[0][:, j, 1, :], sqbnd[nb][:],
                                     start=False, stop=True)
                    nc.scalar.mul(meanb[j][:], p1[:TC, :], 1.0 / WIN)
                    s2b = tmp.tile([TC, EW], dt.float32, tag="s2b")
                    nc.scalar.mul(s2b[:], p2[:TC, :], 1.0 / WIN)
                    msq = tmp.tile([TC, EW], dt.float32, tag="msq")
                    nc.gpsimd.tensor_tensor(msq[:], meanb[j][:], meanb[j][:], OP.mult)
                    var = tmp.tile([TC, EW], dt.float32, tag="var")
                    nc.vector.tensor_tensor(var[:], s2b[:], msq[:], OP.subtract)
                    nc.vector.tensor_scalar_max(var[:], var[:], 0.0)
                    stdf = tmp.tile([TC, EW], dt.float32, tag="stdf")
                    nc.scalar.activation(stdf[:], var[:], AF.Sqrt, bias=eps_t[:TC, 0:1])
                    nc.gpsimd.tensor_copy(stdb[j][:], stdf[:])
                    rstd = tmp.tile([TC, EW], dt.float32, tag="rstd")
                    nc.vector.reciprocal(rstd[:], stdf[:])
                    dlt = tmp.tile([TC, EW], dt.float32, tag="dlt")
                    nc.gpsimd.tensor_tensor(dlt[:], nib[:, j, :], meanb[j][:],
                                            OP.subtract)
                    nrm = tmp.tile([TC, EW], dt.float32, tag="nrm", bufs=3)
                    nc.vector.tensor_tensor(nrm[:], dlt[:], rstd[:], OP.mult)
                    nc.sync.dma_start(out_d.ap()[b, TC * j:TC * (j + 1), e0:e0 + EW],
                                      nrm[:])

                st["meanb"], st["stdb"] = meanb, stdb

        def stage_b2b(st):
                b, e0, xh = st["b"], st["e0"], st["xh"]
                meanb, stdb = st["meanb"], st["stdb"]
                mark("B:mlpp")
                # ---- MLP pred (mean & std paths) ----
                for pi, src in enumerate((meanb, stdb)):
                    h1p = med.tile([128, 2, EW], dt.float16, tag=f"h1p{pi}",
                                   name=f"h1p{pi}")
                    for mi in range(2):
                        p = ps.tile([128, EW], dt.float32, tag="ps")
                        for k in range(NT):
                            nc.tensor.matmul(p[:], W['wp1_t'][:, k, 128 * mi:128 * (mi + 1)],
                                             src[k][:], start=(k == 0),
                                             stop=(k == NT - 1))
                        nc.scalar.activation(h1p[:, mi, :], p[:], AF.Relu,
                                             bias=W['bp1_t'][:, mi:mi + 1])
                    h2p = med.tile([128, 4, EW], dt.float16, tag=f"h2p{pi}",
                                   name=f"h2p{pi}")
                    for mi in range(4):
                        p = ps.tile([128, EW], dt.float32, tag="ps")
                        for k in range(NT):
                            nc.tensor.matmul(p[:], W['wp2x_t'][:, k, 128 * mi:128 * (mi + 1)],
                                             xh[:, k, :], start=(k == 0), stop=False)
                        for c in range(2):
                            nc.tensor.matmul(p[:], W['wp2h_t'][:, c, 128 * mi:128 * (mi + 1)],
                                             h1p[:, c, :], start=False, stop=(c == 1))
                        nc.scalar.activation(h2p[:, mi, :], p[:], AF.Relu,
                                             bias=W['bp2_t'][:, mi:mi + 1])
                    for j in range(NT):
                        p = ps.tile([128, EW], dt.float32, tag="ps")
                        for kc in range(4):
                            nc.tensor.matmul(p[:TC, :], W['wp3_t'][:, kc, j, :],
                                             h2p[:, kc, :], start=(kc == 0),
                                             stop=(kc == 3))
                        o = tmp.tile([TC, EW], dt.float32, tag="op", bufs=2)
                        nc.scalar.activation(o[:], p[:TC, :], AF.Identity,
                                             bias=W['bp3_t'][:, j:j + 1])
                        col = E * (2 + pi)
                        nc.sync.dma_start(
                            out_d.ap()[b, TC * j:TC * (j + 1), col + e0:col + e0 + EW],
                            o[:])

        blocks = [(b, e0) for b in range(BL) for (e0, _) in EH]
        prev = None
        for (b, e0) in blocks:
            if prev is not None:
                stage_b1(prev)
            st = stage_a2(stage_a(b, e0))
            if not c2_t:
                load_consts2()
            load_weights()
            if prev is not None:
                stage_b2(prev)
                stage_b2b(prev)
            prev = st
        stage_b1(prev)
        stage_b2(prev)
        stage_b2b(prev)

    nc.compile()
    return nc


def _prep_inputs(inputs):
    c = _host_constants()
    base = dict(
        chh=c["chh"], chl=c["chl"], shh=c["shh"], shl=c["shl"],
        jrev=c["jrev"], c2=c["c2"], s2=c["s2"], band=c["band"], idf=c["idf"], idh=c["idh"],
        wf1=_f16(inputs["Wf1"]), wf2=_f16(inputs["Wf2"]), wf3=_f16(inputs["Wf3"]),
        wp1=_f16(inputs["Wp1"]), wp2=_f16(inputs["Wp2"]), wp3=_f16(inputs["Wp3"]),
        bf1=np.asarray(inputs["bf1"], np.float32),
        bf2=np.asarray(inputs["bf2"], np.float32),
        bf3=np.asarray(inputs["bf3"], np.float32),
        bp1=np.asarray(inputs["bp1"], np.float32),
        bp2=np.asarray(inputs["bp2"], np.float32),
        bp3=np.asarray(inputs["bp3"], np.float32),
    )
    x = np.ascontiguousarray(np.asarray(inputs["batch_x"], np.float32))
    in_maps = []
    for i in range(NCORES):
        m = dict(base)
        m["x"] = np.ascontiguousarray(x[i * BL:(i + 1) * BL])
        in_maps.append(m)
    return in_maps


def kernel(**inputs):
    from concourse.bass_utils import run_bass_kernel_spmd

    if "nc" not in _cache:
        _cache["nc"] = _build_program()
    nc = _cache["nc"]
    in_maps = _prep_inputs(inputs)
    res = run_bass_kernel_spmd(nc, in_maps, core_ids=list(range(NCORES)))
    _cache["last_result"] = res
    out = np.concatenate([res.results[i]["out"] for i in range(NCORES)], axis=0)
    return out



# revision 2
# speedup vs baseline: 1.0340x; 1.0340x over previous
"""DualAN (normalization) Trainium2 Bass kernel — v2.

kernel(**inputs): FULL inputs (batch_x [32,720,862] f32 + MLP weights), pure
data parallel across 8 NeuronCores ([4,720,862] per core), FULL [32,720,3448]
f32 output.

Per (batch, 431-channel half) block, time-major [t, e] layouts:
  1. x split: xh_s = 1024*fp16(x) (ACT), xl8 = e4m3(1024*(x-xh)) (DVE),
     xh8 = e4m3(x) via gpsimd cast-DMA. All packed for fp8 DoubleRow.
  2. unfolded DFT (K=720): fp16 mains (CH @ xh_s, 1024-scaled psum) + fp8
     DoubleRow corrections ([e4m3(CH)|e4m3(1024 CL)] @ [xl8|xh8]) ->
     fp32-class Xr/Xi for exact top-20 ranking. Evac with scale 1/1024.
  3. mag2 = Xr^2 + Xi^2 (f32); PE-transpose into shared PSUM banks; top-20
     threshold per channel via 3x max8 + 2x in-place match_replace on PSUM.
  4. thr row via PE transposes + f32 outer-product broadcast; mask/masked
     coefs computed f-major (no mask transpose).
  5. iDFT (fp16) -> nib = x - x_filt (fp16); sq = nib^2.
  6. window mean/var via chunk-aligned 3-slab band matmuls (fp16, 1/24
     folded); norm = (nib - mean) * Rsqrt(var + eps).
  7. MLPs in fp8 DoubleRow (K=240/instr): freq-MLP layer 1 reads masked
     coefs through host-precomputed C2@Wf1 (x_filt never materialized for
     the MLP); pred-MLP shares nothing but weights between mean/std paths.
  8. outputs: norm DMA per j; pred trio merged [120,3,431] DMA per j.
"""

import numpy as np
from contextlib import ExitStack

B, S, E = 32, 720, 862
F = 361
FP = 363          # padded to 3*121
PRED = 720
WIN = 24
EPS = 1e-5
NCORES = 8
BL = B // NCORES

TC = 120
NT = 6
EW = 431
FCH = [(0, 121), (121, 121), (242, 121)]
ECH = [(0, 128), (128, 128), (256, 128), (384, 47)]
EH = [(0, 431), (431, 431)]
SC = 1024.0       # hi/lo split scale

_cache = {}


def _f16(a):
    return np.asarray(a).astype(np.float16)


def _f8(a):
    import ml_dtypes
    return np.asarray(a, np.float32).astype(ml_dtypes.float8_e4m3)


def _band_slabs(j):
    """Chunks contributing to window rows of out-chunk j."""
    lo = max(j - 1, 0)
    hi = min(j + 1, NT - 1)
    return list(range(lo, hi + 1))


def _host_constants():
    t = np.arange(S, dtype=np.float64)
    f = np.arange(FP, dtype=np.float64)
    ang = 2.0 * np.pi * np.outer(t, f) / S          # [S, FP]
    C = np.cos(ang)
    Sn = -np.sin(ang)
    C[:, F:] = 0.0
    Sn[:, F:] = 0.0

    def pack_fwd(M):
        # [S, FP] f64 -> mains fp16 [TC, NT, FP], corr fp8 [TC, NT, 2, FP]
        Mh = M.astype(np.float32).astype(np.float16)          # hi
        Ml = (M.astype(np.float32) - Mh.astype(np.float32))   # lo
        mains = np.ascontiguousarray(
            Mh.reshape(NT, TC, FP).transpose(1, 0, 2))
        c8 = np.zeros((TC, NT, 2, FP), np.float32)
        c8[:, :, 0, :] = Mh.astype(np.float32).reshape(NT, TC, FP).transpose(1, 0, 2)
        c8[:, :, 1, :] = (Ml * SC).reshape(NT, TC, FP).transpose(1, 0, 2)
        return mains, _f8(c8)

    CHm, C8 = pack_fwd(C)
    SHm, S8 = pack_fwd(Sn)

    # inverse DFT: x_filt[t] = sum_f c2[f,t] xr[f] + s2[f,t] xi[f]
    w = np.full(FP, 2.0)
    w[0] = 1.0
    w[360] = 1.0
    w[F:] = 0.0
    c2 = (w[:, None] * np.cos(ang.T) / S)           # [FP, S]
    s2 = (w[:, None] * (-np.sin(ang.T)) / S)
    c2[F:] = 0.0
    s2[F:] = 0.0
    c2_t = _f16(c2.reshape(3, 121, S).transpose(1, 0, 2))   # [121, 3, S]
    s2_t = _f16(s2.reshape(3, 121, S).transpose(1, 0, 2))

    # band slab matrices (1/24 folded): [TC(src), 16, TC(out)]
    slab_list = []   # (j, chunk) in emission order
    for j in range(NT):
        for c in _band_slabs(j):
            slab_list.append((j, c))
    band = np.zeros((TC, len(slab_list), TC), np.float64)
    for si, (j, c) in enumerate(slab_list):
        for tt in range(TC):
            g = TC * j + tt
            for q in range(g - WIN // 2, g + WIN // 2):
                qq = min(max(q, 0), S - 1)
                if qq // TC == c:
                    band[qq % TC, si, tt] += 1.0 / WIN
    ident = np.eye(128, dtype=np.float32)
    return dict(
        CH=CHm, SH=SHm, C8=C8, S8=S8, c2=c2_t, s2=s2_t,
        band=_f16(band), slab_list=slab_list, idf=ident,
        ones=np.ones((1, 128), np.float32),
    )


def _prep_weights(inputs):
    """Host-side packing of MLP weights into fp16/fp8 DoubleRow layouts."""
    import ml_dtypes  # noqa: F401
    c = _cache["consts"]
    Wf1 = np.asarray(inputs["Wf1"], np.float32)     # [720, 64]
    Wf2 = np.asarray(inputs["Wf2"], np.float32)     # [784, 128]
    Wf3 = np.asarray(inputs["Wf3"], np.float32)     # [128, 720]
    Wp1 = np.asarray(inputs["Wp1"], np.float32)     # [720, 256]
    Wp2 = np.asarray(inputs["Wp2"], np.float32)     # [976, 512]
    Wp3 = np.asarray(inputs["Wp3"], np.float32)     # [512, 720]

    # W1C/W1S: [FP, 64] = c2 @ Wf1 (fp16 lhsT [121, 3, 64])
    t = np.arange(S, dtype=np.float64)
    f = np.arange(FP, dtype=np.float64)
    ang = 2.0 * np.pi * np.outer(f, t) / S          # [FP, S]
    w = np.full(FP, 2.0); w[0] = 1.0; w[360] = 1.0; w[F:] = 0.0
    c2 = w[:, None] * np.cos(ang) / S
    s2 = w[:, None] * (-np.sin(ang)) / S
    c2[F:] = 0.0; s2[F:] = 0.0
    W1C = (c2 @ Wf1.astype(np.float64)).astype(np.float32)   # [FP, 64]
    W1S = (s2 @ Wf1.astype(np.float64)).astype(np.float32)

    def dr_pack_k(Wk, m):
        # [720, m] -> [TC, 3, 2, m] pairing k-chunks (2t, 2t+1)
        return _f8(Wk.reshape(3, 2, TC, m).transpose(2, 0, 1, 3))

    d = dict(
        w1c=_f16(W1C.reshape(3, 121, 64).transpose(1, 0, 2)),
        w1s=_f16(W1S.reshape(3, 121, 64).transpose(1, 0, 2)),
        wf2x=dr_pack_k(Wf2[64:], 128),
        wf2h=_f8(Wf2[:64]),                          # [64, 128]
        wf3=_f8(Wf3.reshape(2, 64, NT, TC).transpose(1, 0, 2, 3)),  # [64,2,6,120]
        wp1=dr_pack_k(Wp1, 256),
        wp2x=dr_pack_k(Wp2[256:], 512),
        wp2h=_f8(Wp2[:256].reshape(2, 128, 512).transpose(1, 0, 2)),  # [128,2,512]
        wp3=_f8(Wp3.reshape(2, 2, 128, NT, TC).transpose(2, 0, 1, 3, 4)),
        # wp3: [128, pair, slab, 6, 120]: slab s of pair p = kc (2p+s)
        bf1=np.asarray(inputs["bf1"], np.float32).reshape(64, 1),
        bf2=np.asarray(inputs["bf2"], np.float32).reshape(128, 1),
        bf3=np.asarray(inputs["bf3"], np.float32).reshape(NT, TC).T.copy(),
        bp1=np.asarray(inputs["bp1"], np.float32).reshape(2, 128).T.copy(),
        bp2=np.asarray(inputs["bp2"], np.float32).reshape(4, 128).T.copy(),
        bp3=np.asarray(inputs["bp3"], np.float32).reshape(NT, TC).T.copy(),
    )
    return d


def _build_program():
    import concourse.tile as tile
    from concourse import bacc, mybir

    dt = mybir.dt
    AF = mybir.ActivationFunctionType
    OP = mybir.AluOpType
    DR = mybir.MatmulPerfMode.DoubleRow
    ZB = _cache.get("zero_bias", False)
    c = _cache["consts"]
    slab_list = c["slab_list"]

    nc = bacc.Bacc("TRN2", target_bir_lowering=False, debug=False)

    x_d = nc.dram_tensor("x", [BL, S, E], dt.float32, kind="ExternalInput")
    CH_d = nc.dram_tensor("CH", [TC, NT, FP], dt.float16, kind="ExternalInput")
    SH_d = nc.dram_tensor("SH", [TC, NT, FP], dt.float16, kind="ExternalInput")
    C8_d = nc.dram_tensor("C8", [TC, NT, 2, FP], dt.float8e4, kind="ExternalInput")
    S8_d = nc.dram_tensor("S8", [TC, NT, 2, FP], dt.float8e4, kind="ExternalInput")
    c2_d = nc.dram_tensor("c2", [121, 3, S], dt.float16, kind="ExternalInput")
    s2_d = nc.dram_tensor("s2", [121, 3, S], dt.float16, kind="ExternalInput")
    band_d = nc.dram_tensor("band", [TC, len(slab_list), TC], dt.float16,
                            kind="ExternalInput")
    idf_d = nc.dram_tensor("idf", [128, 128], dt.float32, kind="ExternalInput")
    ones_d = nc.dram_tensor("ones", [1, 128], dt.float32, kind="ExternalInput")
    w1c_d = nc.dram_tensor("w1c", [121, 3, 64], dt.float16, kind="ExternalInput")
    w1s_d = nc.dram_tensor("w1s", [121, 3, 64], dt.float16, kind="ExternalInput")
    wf2x_d = nc.dram_tensor("wf2x", [TC, 3, 2, 128], dt.float8e4, kind="ExternalInput")
    wf2h_d = nc.dram_tensor("wf2h", [64, 128], dt.float8e4, kind="ExternalInput")
    wf3_d = nc.dram_tensor("wf3", [64, 2, NT, TC], dt.float8e4, kind="ExternalInput")
    wp1_d = nc.dram_tensor("wp1", [TC, 3, 2, 256], dt.float8e4, kind="ExternalInput")
    wp2x_d = nc.dram_tensor("wp2x", [TC, 3, 2, 512], dt.float8e4, kind="ExternalInput")
    wp2h_d = nc.dram_tensor("wp2h", [128, 2, 512], dt.float8e4, kind="ExternalInput")
    wp3_d = nc.dram_tensor("wp3", [128, 2, 2, NT, TC], dt.float8e4, kind="ExternalInput")
    bf1_d = nc.dram_tensor("bf1", [64, 1], dt.float32, kind="ExternalInput")
    bf2_d = nc.dram_tensor("bf2", [128, 1], dt.float32, kind="ExternalInput")
    bf3_d = nc.dram_tensor("bf3", [TC, NT], dt.float32, kind="ExternalInput")
    bp1_d = nc.dram_tensor("bp1", [128, 2], dt.float32, kind="ExternalInput")
    bp2_d = nc.dram_tensor("bp2", [128, 4], dt.float32, kind="ExternalInput")
    bp3_d = nc.dram_tensor("bp3", [TC, NT], dt.float32, kind="ExternalInput")
    out_d = nc.dram_tensor("out", [BL, S, 4 * E], dt.float32, kind="ExternalOutput")

    with tile.TileContext(nc) as tc_, ExitStack() as ctx:
        const = ctx.enter_context(tc_.tile_pool(name="const", bufs=1))
        big = ctx.enter_context(tc_.tile_pool(name="big", bufs=1))
        tmp = ctx.enter_context(tc_.tile_pool(name="tmp", bufs=1))
        ps1 = ctx.enter_context(tc_.tile_pool(name="ps1", bufs=1, space="PSUM"))
        ps2 = ctx.enter_context(tc_.tile_pool(name="ps2", bufs=1, space="PSUM"))

        def cload(d, shape, dtype, name):
            t_ = const.tile(shape, dtype, name=name)
            nc.sync.dma_start(t_[:], d.ap()[:])
            return t_

        CH_t = cload(CH_d, [TC, NT, FP], dt.float16, "CH")
        SH_t = cload(SH_d, [TC, NT, FP], dt.float16, "SH")
        C8_t = cload(C8_d, [TC, NT, 2, FP], dt.float8e4, "C8")
        S8_t = cload(S8_d, [TC, NT, 2, FP], dt.float8e4, "S8")
        c2_t = cload(c2_d, [121, 3, S], dt.float16, "c2")
        s2_t = cload(s2_d, [121, 3, S], dt.float16, "s2")
        band_t = cload(band_d, [TC, len(slab_list), TC], dt.float16, "band")
        idf_t = cload(idf_d, [128, 128], dt.float32, "idf")
        ones_t = cload(ones_d, [1, 128], dt.float32, "ones")
        w1c_t = cload(w1c_d, [121, 3, 64], dt.float16, "w1c")
        w1s_t = cload(w1s_d, [121, 3, 64], dt.float16, "w1s")
        wf2x_t = cload(wf2x_d, [TC, 3, 2, 128], dt.float8e4, "wf2x")
        wf2h_t = cload(wf2h_d, [64, 128], dt.float8e4, "wf2h")
        wf3_t = cload(wf3_d, [64, 2, NT, TC], dt.float8e4, "wf3")
        wp1_t = cload(wp1_d, [TC, 3, 2, 256], dt.float8e4, "wp1")
        wp2x_t = cload(wp2x_d, [TC, 3, 2, 512], dt.float8e4, "wp2x")
        wp2h_t = cload(wp2h_d, [128, 2, 512], dt.float8e4, "wp2h")
        wp3_t = cload(wp3_d, [128, 2, 2, NT, TC], dt.float8e4, "wp3")
        bf1_t = cload(bf1_d, [64, 1], dt.float32, "bf1")
        bf2_t = cload(bf2_d, [128, 1], dt.float32, "bf2")
        bf3_t = cload(bf3_d, [TC, NT], dt.float32, "bf3")
        bp1_t = cload(bp1_d, [128, 2], dt.float32, "bp1")
        bp2_t = cload(bp2_d, [128, 4], dt.float32, "bp2")
        bp3_t = cload(bp3_d, [TC, NT], dt.float32, "bp3")
        eps_t = const.tile([128, 1], dt.float32, name="eps")
        nc.vector.memset(eps_t[:], EPS)

        def block(b, e0):
            # ---- load + split ----
            x32 = big.tile([TC, NT, EW], dt.float32, tag="x32", bufs=2)
            nc.sync.dma_start(
                x32[:], x_d.ap()[b, :, e0:e0 + EW].rearrange(
                    "(c p) e -> p c e", p=TC))
            xh = big.tile([TC, NT, EW], dt.float16, tag="xh", bufs=2)
            nc.scalar.activation(xh[:], x32[:], AF.Identity, scale=SC)
            x8 = big.tile([TC, NT, 2, EW], dt.float8e4, tag="x8", bufs=2)
            nc.vector.scalar_tensor_tensor(
                x8[:, :, 0, :], x32[:], SC, xh[:], OP.mult, OP.subtract)
            nc.gpsimd.dma_start(x8[:, :, 1, :], x32[:])

            # ---- DFT: mains fp16 + corrections fp8 DR ----
            xr_t = big.tile([121, 3, EW], dt.float32, tag="xr", bufs=1)
            xi_t = big.tile([121, 3, EW], dt.float32, tag="xi", bufs=1)
            for mats, m8, dst in ((CH_t, C8_t, xr_t), (SH_t, S8_t, xi_t)):
                for ci, (f0, fw) in enumerate(FCH):
                    p = ps1.tile([128, 512], dt.float32, tag="psA", bufs=3)
                    for k in range(NT):
                        nc.tensor.matmul(p[0:fw, 0:EW], mats[:, k, f0:f0 + fw],
                                         xh[:, k, :], start=(k == 0), stop=False)
                    for k in range(NT):
                        nc.tensor.matmul(p[0:fw, 0:EW], m8[:, k, :, f0:f0 + fw],
                                         x8[:, k, :, :], start=False,
                                         stop=(k == NT - 1), perf_mode=DR)
                    nc.scalar.activation(dst[:, ci, :], p[0:121, 0:EW],
                                         AF.Identity, scale=1.0 / SC)

            # ---- mag2 (f32) ----
            sqr = tmp.tile([121, 3, EW], dt.float32, tag="sqr", bufs=1)
            nc.scalar.square(sqr[:], xr_t[:])
            sqi = tmp.tile([121, 3, EW], dt.float32, tag="sqi", bufs=1)
            nc.scalar.square(sqi[:], xi_t[:])
            mag2 = big.tile([121, 3, EW], dt.float32, tag="mag2", bufs=1)
            nc.vector.tensor_tensor(mag2[:], sqr[:], sqi[:], OP.add)

            # ---- transpose chunks into PSUM + top-20 threshold ----
            pthr = ps1.tile([128, 512], dt.float32, tag="psTH", bufs=1)
            m3s = []
            for ei, (ee0, ew) in enumerate(ECH):
                pt = ps1.tile([128, 512], dt.float32, tag="psA", bufs=3)
                for ci, (f0, fw) in enumerate(FCH):
                    nc.tensor.matmul(pt[0:ew, f0:f0 + fw],
                                     mag2[0:fw, ci, ee0:ee0 + ew],
                                     idf_t[0:fw, 0:fw], is_transpose=True,
                                     start=(ci == 0), stop=(ci == 2))
                m1 = tmp.tile([128, 8], dt.float32, tag=f"m1_{ei}")
                nc.vector.max(m1[0:ew, :], pt[0:ew, 0:FP])
                nc.vector.match_replace(pt[0:ew, 0:FP], m1[0:ew, :],
                                        pt[0:ew, 0:FP], -1e30)
                m2 = tmp.tile([128, 8], dt.float32, tag=f"m2_{ei}")
                nc.vector.max(m2[0:ew, :], pt[0:ew, 0:FP])
                nc.vector.match_replace(pt[0:ew, 0:FP], m2[0:ew, :],
                                        pt[0:ew, 0:FP], -1e30)
                m3 = tmp.tile([128, 8], dt.float32, tag=f"m3_{ei}")
                nc.vector.max(m3[0:ew, :], pt[0:ew, 0:FP])
                m3s.append(m3)
            for ei, (ee0, ew) in enumerate(ECH):
                nc.tensor.matmul(pthr[0:1, ee0:ee0 + ew], m3s[ei][0:ew, 3:4],
                                 idf_t[0:ew, 0:ew], is_transpose=True,
                                 start=(ei == 0), stop=(ei == 3))
            thr_row = tmp.tile([1, EW], dt.float32, tag="thr_row", bufs=2)
            nc.vector.tensor_copy(thr_row[:], pthr[0:1, 0:EW])
            ptb = ps1.tile([128, 512], dt.float32, tag="psTB", bufs=1)
            nc.tensor.matmul(ptb[:, 0:EW], ones_t[:], thr_row[:],
                             start=True, stop=True)

            # ---- mask + masked coefs (f-major) ----
            mask = big.tile([121, 3, EW], dt.float16, tag="mask", bufs=1)
            for ci in range(3):
                nc.vector.tensor_tensor(mask[:, ci, :], mag2[:, ci, :],
                                        ptb[0:121, 0:EW], OP.is_ge)
            xrm = big.tile([121, 3, EW], dt.float16, tag="xrm", bufs=2)
            nc.vector.tensor_tensor(xrm[:], xr_t[:], mask[:], OP.mult)
            xim = big.tile([121, 3, EW], dt.float16, tag="xim", bufs=2)
            nc.gpsimd.tensor_tensor(xim[:], xi_t[:], mask[:], OP.mult)

            # ---- iDFT -> nib (fp16), sq ----
            nib = big.tile([TC, NT, EW], dt.float16, tag="nib", bufs=2)
            for j in range(NT):
                t0 = TC * j
                p = ps1.tile([128, 512], dt.float32, tag="psA", bufs=3)
                for ci in range(3):
                    nc.tensor.matmul(p[0:TC, 0:EW], c2_t[:, ci, t0:t0 + TC],
                                     xrm[:, ci, :], start=(ci == 0), stop=False)
                    nc.tensor.matmul(p[0:TC, 0:EW], s2_t[:, ci, t0:t0 + TC],
                                     xim[:, ci, :], start=False, stop=(ci == 2))
                eng = nc.vector if j % 2 == 0 else nc.gpsimd
                eng.scalar_tensor_tensor(nib[:, j, :], p[0:TC, 0:EW], -1.0,
                                         x32[:, j, :], OP.mult, OP.add)
            sq = big.tile([TC, NT, EW], dt.float16, tag="sq", bufs=1)
            nc.vector.tensor_tensor(sq[:], nib[:], nib[:], OP.mult)

            # ---- MLP freq ----
            p = ps1.tile([128, 512], dt.float32, tag="psA", bufs=3)
            for ci in range(3):
                nc.tensor.matmul(p[0:64, 0:EW], w1c_t[:, ci, :], xrm[:, ci, :],
                                 start=(ci == 0), stop=False)
                nc.tensor.matmul(p[0:64, 0:EW], w1s_t[:, ci, :], xim[:, ci, :],
                                 start=False, stop=(ci == 2))
            h1f = tmp.tile([64, EW], dt.float8e4, tag="h1f", bufs=2)
            nc.scalar.activation(h1f[:], p[0:64, 0:EW], AF.Relu, bias=bf1_t[0:64, :])
            p = ps1.tile([128, 512], dt.float32, tag="psA", bufs=3)
            for tpair in range(3):
                nc.tensor.matmul(p[:, 0:EW], wf2x_t[:, tpair, :, :],
                                 x8[:, 2 * tpair:2 * tpair + 2, 1, :],
                                 start=(tpair == 0), stop=False, perf_mode=DR)
            nc.tensor.matmul(p[:, 0:EW], wf2h_t[:], h1f[:], start=False, stop=True)
            h2f = tmp.tile([128, EW], dt.float8e4, tag="h2f", bufs=2)
            nc.scalar.activation(h2f[:], p[:, 0:EW], AF.Relu, bias=bf2_t[:])

            # ---- band stats + norm ----
            mean16 = big.tile([TC, NT, EW], dt.float16, tag="mean16", bufs=1)
            std8 = big.tile([TC, NT, EW], dt.float8e4, tag="std8", bufs=2)
            orow = out_d.ap()[b, :, :].rearrange("t (s e) -> t s e", e=E)
            si = 0
            for j in range(NT):
                chunks = _band_slabs(j)
                pp = ps2.tile([128, 2, 512], dt.float32, tag="psBD")
                for k, cch in enumerate(chunks):
                    nc.tensor.matmul(pp[0:TC, 0, 0:EW], band_t[:, si + k, :],
                                     nib[:, cch, :], start=(k == 0),
                                     stop=(k == len(chunks) - 1))
                for k, cch in enumerate(chunks):
                    nc.tensor.matmul(pp[0:TC, 1, 0:EW], band_t[:, si + k, :],
                                     sq[:, cch, :], start=(k == 0),
                                     stop=(k == len(chunks) - 1))
                si += len(chunks)
                nc.scalar.copy(mean16[:, j, :], pp[0:TC, 0, 0:EW])
                msq = tmp.tile([TC, EW], dt.float16, tag="msq", bufs=2)
                nc.vector.tensor_tensor(msq[:], mean16[:, j, :],
                                        mean16[:, j, :], OP.mult)
                var16 = tmp.tile([TC, EW], dt.float16, tag="var16", bufs=2)
                nc.vector.tensor_tensor(var16[:], pp[0:TC, 1, 0:EW], msq[:],
                                        OP.subtract)
                std16 = tmp.tile([TC, EW], dt.float16, tag="std16", bufs=2)
                nc.scalar.activation(std16[:], var16[:], AF.Sqrt,
                                     bias=eps_t[0:TC, :])
                nc.scalar.activation(std8[:, j, :], var16[:], AF.Sqrt,
                                     bias=eps_t[0:TC, :])
                delta = tmp.tile([TC, EW], dt.float16, tag="delta", bufs=2)
                nc.vector.tensor_tensor(delta[:], nib[:, j, :], mean16[:, j, :],
                                        OP.subtract)
                norm = tmp.tile([TC, EW], dt.float32, tag="norm", bufs=3)
                nc.gpsimd.tensor_tensor(norm[:], delta[:], std16[:], OP.divide)
                nc.sync.dma_start(orow[TC * j:TC * (j + 1), 0, e0:e0 + EW],
                                  norm[:])
            mean8 = big.tile([TC, NT, EW], dt.float8e4, tag="mean8", bufs=2)
            nc.gpsimd.dma_start(mean8[:], mean16[:])

            # ---- MLP pred layers 1-2 (mean & std paths) ----
            h2ps = []
            for pi, src in enumerate((mean8, std8)):
                pq = ps2.tile([128, 2, 512], dt.float32, tag="psBD")
                for mi in range(2):
                    for tpair in range(3):
                        nc.tensor.matmul(
                            pq[:, mi, 0:EW],
                            wp1_t[:, tpair, :, 128 * mi:128 * (mi + 1)],
                            src[:, 2 * tpair:2 * tpair + 2, :],
                            start=(tpair == 0), stop=(tpair == 2), perf_mode=DR)
                h1p = tmp.tile([128, 2, EW], dt.float8e4, tag=f"h1p{pi}", bufs=2)
                for mi in range(2):
                    nc.scalar.activation(h1p[:, mi, :], pq[:, mi, 0:EW], AF.Relu,
                                         bias=bp1_t[:, mi:mi + 1])
                h2p = big.tile([128, 4, EW], dt.float8e4, tag=f"h2p{pi}", bufs=2)
                for half in range(2):
                    pr = ps2.tile([128, 2, 512], dt.float32, tag="psBD")
                    for mi2 in range(2):
                        mi = 2 * half + mi2
                        for tpair in range(3):
                            nc.tensor.matmul(
                                pr[:, mi2, 0:EW],
                                wp2x_t[:, tpair, :, 128 * mi:128 * (mi + 1)],
                                x8[:, 2 * tpair:2 * tpair + 2, 1, :],
                                start=(tpair == 0), stop=False, perf_mode=DR)
                        nc.tensor.matmul(pr[:, mi2, 0:EW],
                                         wp2h_t[:, :, 128 * mi:128 * (mi + 1)],
                                         h1p[:], start=False, stop=True,
                                         perf_mode=DR)
                        nc.scalar.activation(h2p[:, mi, :], pr[:, mi2, 0:EW],
                                             AF.Relu, bias=bp2_t[:, mi:mi + 1])
                h2ps.append(h2p)

            # ---- final layers, fused per-j, merged trio DMA ----
            for j in range(NT):
                pa = ps1.tile([128, 512], dt.float32, tag="psA", bufs=3)
                nc.tensor.matmul(pa[0:TC, 0:EW], wf3_t[:, j, :], h2f[:],
                                 start=True, stop=True)
                pb = ps1.tile([128, 512], dt.float32, tag="psA", bufs=3)
                for pr_ in range(2):
                    nc.tensor.matmul(pb[0:TC, 0:EW], wp3_t[:, pr_, :, j, :],
                                     h2ps[0][:, 2 * pr_:2 * pr_ + 2, :],
                                     start=(pr_ == 0), stop=(pr_ == 1),
                                     perf_mode=DR)
                pc = ps1.tile([128, 512], dt.float32, tag="psA", bufs=3)
                for pr_ in range(2):
                    nc.tensor.matmul(pc[0:TC, 0:EW], wp3_t[:, pr_, :, j, :],
                                     h2ps[1][:, 2 * pr_:2 * pr_ + 2, :],
                                     start=(pr_ == 0), stop=(pr_ == 1),
                                     perf_mode=DR)
                trio = tmp.tile([TC, 3, EW], dt.float32, tag="trio", bufs=2)
                nc.scalar.activation(trio[:, 0, :], pa[0:TC, 0:EW], AF.Identity,
                                     bias=bf3_t[:, j:j + 1])
                nc.vector.tensor_scalar(trio[:, 1, :], pb[0:TC, 0:EW],
                                        bp3_t[:, j:j + 1], None, OP.add)
                nc.gpsimd.tensor_scalar(trio[:, 2, :], pc[0:TC, 0:EW],
                                        bp3_t[:, j:j + 1], None, OP.add)
                nc.sync.dma_start(orow[TC * j:TC * (j + 1), 1:4, e0:e0 + EW],
                                  trio[:])

        for b in range(BL):
            for (e0, _) in EH:
                block(b, e0)

    nc.compile()
    return nc


def _prep_inputs(inputs):
    c = _cache["consts"]
    w = _prep_weights(inputs)
    base = dict(
        CH=c["CH"], SH=c["SH"], C8=c["C8"], S8=c["S8"], c2=c["c2"], s2=c["s2"],
        band=c["band"], idf=c["idf"], ones=c["ones"], **w)
    x = np.ascontiguousarray(np.asarray(inputs["batch_x"], np.float32))
    in_maps = []
    for i in range(NCORES):
        m = dict(base)
        m["x"] = np.ascontiguousarray(x[i * BL:(i + 1) * BL])
        in_maps.append(m)
    return in_maps


def kernel(**inputs):
    from concourse.bass_utils import run_bass_kernel_spmd

    if "consts" not in _cache:
        _cache["consts"] = _host_constants()
    _cache["zero_bias"] = all(
        not np.any(np.asarray(inputs[k]))
        for k in ("bf1", "bf2", "bf3", "bp1", "bp2", "bp3"))
    if "nc" not in _cache:
        _cache["nc"] = _build_program()
    nc = _cache["nc"]
    in_maps = _prep_inputs(inputs)
    res = run_bass_kernel_spmd(nc, in_maps, core_ids=list(range(NCORES)))
    _cache["last_result"] = res
    out = np.concatenate([res.results[i]["out"] for i in range(NCORES)], axis=0)
    return out


# revision 4
# speedup vs baseline: 1.0499x; 1.0155x over previous
"""DualAN (normalization) Trainium2 Bass kernel — v2.

kernel(**inputs): FULL inputs (batch_x [32,720,862] f32 + MLP weights), pure
data parallel across 8 NeuronCores ([4,720,862] per core), FULL [32,720,3448]
f32 output.

Per (batch, 431-channel half) block, time-major [t, e] layouts:
  1. x split: xh_s = 1024*fp16(x) (ACT), xl8 = e4m3(1024*(x-xh)) (DVE),
     xh8 = e4m3(x) via gpsimd cast-DMA. All packed for fp8 DoubleRow.
  2. unfolded DFT (K=720): fp16 mains (CH @ xh_s, 1024-scaled psum) + fp8
     DoubleRow corrections ([e4m3(CH)|e4m3(1024 CL)] @ [xl8|xh8]) ->
     fp32-class Xr/Xi for exact top-20 ranking. Evac with scale 1/1024.
  3. mag2 = Xr^2 + Xi^2 (f32); PE-transpose into shared PSUM banks; top-20
     threshold per channel via 3x max8 + 2x in-place match_replace on PSUM.
  4. thr row via PE transposes + f32 outer-product broadcast; mask/masked
     coefs computed f-major (no mask transpose).
  5. iDFT (fp16) -> nib = x - x_filt (fp16); sq = nib^2.
  6. window mean/var via chunk-aligned 3-slab band matmuls (fp16, 1/24
     folded); norm = (nib - mean) * Rsqrt(var + eps).
  7. MLPs in fp8 DoubleRow (K=240/instr): freq-MLP layer 1 reads masked
     coefs through host-precomputed C2@Wf1 (x_filt never materialized for
     the MLP); pred-MLP shares nothing but weights between mean/std paths.
  8. outputs: norm DMA per j; pred trio merged [120,3,431] DMA per j.
"""

import numpy as np
from contextlib import ExitStack

B, S, E = 32, 720, 862
F = 361
FP = 363          # padded to 3*121
FP8 = 368         # fp8 DR weight slab stride (16B aligned)
PRED = 720
WIN = 24
EPS = 1e-5
NCORES = 8
BL = B // NCORES

TC = 120
NT = 6
EW = 431
FCH = [(0, 121), (121, 121), (242, 121)]
ECH = [(0, 128), (128, 128), (256, 128), (384, 47)]
EH = [(0, 431), (431, 431)]
SC = 1024.0       # hi/lo split scale
W8 = 64.0         # fp8 weight scale
M8 = 4.0          # fp8 mean/std scale

_cache = {}


def _f16(a):
    return np.asarray(a).astype(np.float16)


def _f8(a):
    import ml_dtypes
    return np.asarray(a, np.float32).astype(ml_dtypes.float8_e4m3)


def _band_slabs(j):
    """Chunks contributing to window rows of out-chunk j."""
    lo = max(j - 1, 0)
    hi = min(j + 1, NT - 1)
    return list(range(lo, hi + 1))


def _host_constants():
    t = np.arange(S, dtype=np.float64)
    f = np.arange(FP, dtype=np.float64)
    ang = 2.0 * np.pi * np.outer(t, f) / S          # [S, FP]
    C = np.cos(ang)
    Sn = -np.sin(ang)
    C[:, F:] = 0.0
    Sn[:, F:] = 0.0

    def pack_fwd(M):
        # [S, FP] f64 -> mains fp16 [TC, NT, FP], corr fp8 [TC, NT, 2, FP]
        Mh = M.astype(np.float32).astype(np.float16)          # hi
        Ml = (M.astype(np.float32) - Mh.astype(np.float32))   # lo
        mains = np.ascontiguousarray(
            Mh.reshape(NT, TC, FP).transpose(1, 0, 2))
        c8 = np.zeros((TC, NT, 2, FP8), np.float32)
        c8[:, :, 0, :FP] = Mh.astype(np.float32).reshape(NT, TC, FP).transpose(1, 0, 2)
        c8[:, :, 1, :FP] = (Ml * SC).reshape(NT, TC, FP).transpose(1, 0, 2)
        return mains, _f8(c8)

    CHm, C8 = pack_fwd(C)
    SHm, S8 = pack_fwd(Sn)

    # inverse DFT: x_filt[t] = sum_f c2[f,t] xr[f] + s2[f,t] xi[f]
    w = np.full(FP, 2.0)
    w[0] = 1.0
    w[360] = 1.0
    w[F:] = 0.0
    c2 = (w[:, None] * np.cos(ang.T) / S)           # [FP, S]
    s2 = (w[:, None] * (-np.sin(ang.T)) / S)
    c2[F:] = 0.0
    s2[F:] = 0.0
    c2_t = _f16(c2.reshape(3, 121, S).transpose(1, 0, 2))   # [121, 3, S]
    s2_t = _f16(s2.reshape(3, 121, S).transpose(1, 0, 2))

    # band slab matrices (1/24 folded): [TC(src), 16, TC(out)]
    slab_list = []   # (j, chunk) in emission order
    for j in range(NT):
        for c in _band_slabs(j):
            slab_list.append((j, c))
    band = np.zeros((TC, len(slab_list), TC), np.float64)
    for si, (j, c) in enumerate(slab_list):
        for tt in range(TC):
            g = TC * j + tt
            for q in range(g - WIN // 2, g + WIN // 2):
                qq = min(max(q, 0), S - 1)
                if qq // TC == c:
                    band[qq % TC, si, tt] += 1.0 / WIN
    ident = np.eye(128, dtype=np.float32)
    return dict(
        CH=CHm, SH=SHm, C8=C8, S8=S8, c2=c2_t, s2=s2_t,
        band=_f16(band), slab_list=slab_list, idf=ident,
        ones=np.ones((1, 128), np.float32),
    )


def _prep_weights(inputs):
    """Host-side packing of MLP weights into fp16/fp8 DoubleRow layouts."""
    import ml_dtypes  # noqa: F401
    c = _cache["consts"]
    Wf1 = np.asarray(inputs["Wf1"], np.float32)     # [720, 64]
    Wf2 = np.asarray(inputs["Wf2"], np.float32)     # [784, 128]
    Wf3 = np.asarray(inputs["Wf3"], np.float32)     # [128, 720]
    Wp1 = np.asarray(inputs["Wp1"], np.float32)     # [720, 256]
    Wp2 = np.asarray(inputs["Wp2"], np.float32)     # [976, 512]
    Wp3 = np.asarray(inputs["Wp3"], np.float32)     # [512, 720]

    # W1C/W1S: [FP, 64] = c2 @ Wf1 (fp16 lhsT [121, 3, 64])
    t = np.arange(S, dtype=np.float64)
    f = np.arange(FP, dtype=np.float64)
    ang = 2.0 * np.pi * np.outer(f, t) / S          # [FP, S]
    w = np.full(FP, 2.0); w[0] = 1.0; w[360] = 1.0; w[F:] = 0.0
    c2 = w[:, None] * np.cos(ang) / S
    s2 = w[:, None] * (-np.sin(ang)) / S
    c2[F:] = 0.0; s2[F:] = 0.0
    W1C = (c2 @ Wf1.astype(np.float64)).astype(np.float32)   # [FP, 64]
    W1S = (s2 @ Wf1.astype(np.float64)).astype(np.float32)

    def dr_pack_k(Wk, m):
        # [720, m] -> [TC, 3, 2, m] pairing k-chunks (2t, 2t+1)
        return _f8(W8 * Wk.reshape(3, 2, TC, m).transpose(2, 0, 1, 3))

    d = dict(
        w1c=_f16(W1C.reshape(3, 121, 64).transpose(1, 0, 2)),
        w1s=_f16(W1S.reshape(3, 121, 64).transpose(1, 0, 2)),
        wf2x=dr_pack_k(Wf2[64:], 128),
        wf2h=_f8(W8 * Wf2[:64]),                     # [64, 128]
        wf3=_f8(W8 * Wf3.reshape(2, 64, NT, TC).transpose(1, 0, 2, 3)),
        wp1=dr_pack_k(Wp1, 256),
        wp2x=dr_pack_k(Wp2[256:], 512),
        wp2h=_f8(W8 * Wp2[:256].reshape(2, 128, 512).transpose(1, 0, 2)),
        wp3=_f8(W8 * Wp3.reshape(2, 2, 128, NT, TC).transpose(2, 0, 1, 3, 4)),
        # wp3: [128, pair, slab, 6, 120]: slab s of pair p = kc (2p+s)
        bf1=np.asarray(inputs["bf1"], np.float32).reshape(64, 1),
        bf2=np.asarray(inputs["bf2"], np.float32).reshape(128, 1),
        bf3=np.asarray(inputs["bf3"], np.float32).reshape(NT, TC).T.copy(),
        bp1=np.asarray(inputs["bp1"], np.float32).reshape(2, 128).T.copy(),
        bp2=np.asarray(inputs["bp2"], np.float32).reshape(4, 128).T.copy(),
        bp3=np.asarray(inputs["bp3"], np.float32).reshape(NT, TC).T.copy(),
    )
    return d


def _build_program():
    import concourse.tile as tile
    from concourse import bacc, mybir

    dt = mybir.dt
    AF = mybir.ActivationFunctionType
    OP = mybir.AluOpType
    DR = mybir.MatmulPerfMode.DoubleRow
    ZB = _cache.get("zero_bias", False)
    c = _cache["consts"]
    slab_list = c["slab_list"]

    nc = bacc.Bacc("TRN2", target_bir_lowering=False, debug=False)

    x_d = nc.dram_tensor("x", [BL, S, E], dt.float32, kind="ExternalInput")
    CH_d = nc.dram_tensor("CH", [TC, NT, FP], dt.float16, kind="ExternalInput")
    SH_d = nc.dram_tensor("SH", [TC, NT, FP], dt.float16, kind="ExternalInput")
    C8_d = nc.dram_tensor("C8", [TC, NT, 2, FP8], dt.float8e4, kind="ExternalInput")
    S8_d = nc.dram_tensor("S8", [TC, NT, 2, FP8], dt.float8e4, kind="ExternalInput")
    c2_d = nc.dram_tensor("c2", [121, 3, S], dt.float16, kind="ExternalInput")
    s2_d = nc.dram_tensor("s2", [121, 3, S], dt.float16, kind="ExternalInput")
    band_d = nc.dram_tensor("band", [TC, len(slab_list), TC], dt.float16,
                            kind="ExternalInput")
    idf_d = nc.dram_tensor("idf", [128, 128], dt.float32, kind="ExternalInput")
    ones_d = nc.dram_tensor("ones", [1, 128], dt.float32, kind="ExternalInput")
    w1c_d = nc.dram_tensor("w1c", [121, 3, 64], dt.float16, kind="ExternalInput")
    w1s_d = nc.dram_tensor("w1s", [121, 3, 64], dt.float16, kind="ExternalInput")
    wf2x_d = nc.dram_tensor("wf2x", [TC, 3, 2, 128], dt.float8e4, kind="ExternalInput")
    wf2h_d = nc.dram_tensor("wf2h", [64, 128], dt.float8e4, kind="ExternalInput")
    wf3_d = nc.dram_tensor("wf3", [64, 2, NT, TC], dt.float8e4, kind="ExternalInput")
    wp1_d = nc.dram_tensor("wp1", [TC, 3, 2, 256], dt.float8e4, kind="ExternalInput")
    wp2x_d = nc.dram_tensor("wp2x", [TC, 3, 2, 512], dt.float8e4, kind="ExternalInput")
    wp2h_d = nc.dram_tensor("wp2h", [128, 2, 512], dt.float8e4, kind="ExternalInput")
    wp3_d = nc.dram_tensor("wp3", [128, 2, 2, NT, TC], dt.float8e4, kind="ExternalInput")
    bf1_d = nc.dram_tensor("bf1", [64, 1], dt.float32, kind="ExternalInput")
    bf2_d = nc.dram_tensor("bf2", [128, 1], dt.float32, kind="ExternalInput")
    bf3_d = nc.dram_tensor("bf3", [TC, NT], dt.float32, kind="ExternalInput")
    bp1_d = nc.dram_tensor("bp1", [128, 2], dt.float32, kind="ExternalInput")
    bp2_d = nc.dram_tensor("bp2", [128, 4], dt.float32, kind="ExternalInput")
    bp3_d = nc.dram_tensor("bp3", [TC, NT], dt.float32, kind="ExternalInput")
    out_d = nc.dram_tensor("out", [BL, S, 4 * E], dt.float32, kind="ExternalOutput")

    with tile.TileContext(nc) as tc_, ExitStack() as ctx:
        const = ctx.enter_context(tc_.tile_pool(name="const", bufs=1))
        big = ctx.enter_context(tc_.tile_pool(name="big", bufs=1))
        tmp = ctx.enter_context(tc_.tile_pool(name="tmp", bufs=1))
        ps1 = ctx.enter_context(tc_.tile_pool(name="ps1", bufs=1, space="PSUM"))
        ps2 = ctx.enter_context(tc_.tile_pool(name="ps2", bufs=1, space="PSUM"))

        def cload(d, shape, dtype, name):
            t_ = const.tile(shape, dtype, name=name)
            nc.sync.dma_start(t_[:], d.ap()[:])
            return t_

        CH_t = cload(CH_d, [TC, NT, FP], dt.float16, "CH")
        SH_t = cload(SH_d, [TC, NT, FP], dt.float16, "SH")
        C8_t = cload(C8_d, [TC, NT, 2, FP8], dt.float8e4, "C8")
        S8_t = cload(S8_d, [TC, NT, 2, FP8], dt.float8e4, "S8")
        c2_t = cload(c2_d, [121, 3, S], dt.float16, "c2")
        s2_t = cload(s2_d, [121, 3, S], dt.float16, "s2")
        band_t = cload(band_d, [TC, len(slab_list), TC], dt.float16, "band")
        idf_t = cload(idf_d, [128, 128], dt.float32, "idf")
        ones_t = cload(ones_d, [1, 128], dt.float32, "ones")
        w1c_t = cload(w1c_d, [121, 3, 64], dt.float16, "w1c")
        w1s_t = cload(w1s_d, [121, 3, 64], dt.float16, "w1s")
        wf2x_t = cload(wf2x_d, [TC, 3, 2, 128], dt.float8e4, "wf2x")
        wf2h_t = cload(wf2h_d, [64, 128], dt.float8e4, "wf2h")
        wf3_t = cload(wf3_d, [64, 2, NT, TC], dt.float8e4, "wf3")
        wp1_t = cload(wp1_d, [TC, 3, 2, 256], dt.float8e4, "wp1")
        wp2x_t = cload(wp2x_d, [TC, 3, 2, 512], dt.float8e4, "wp2x")
        wp2h_t = cload(wp2h_d, [128, 2, 512], dt.float8e4, "wp2h")
        wp3_t = cload(wp3_d, [128, 2, 2, NT, TC], dt.float8e4, "wp3")
        bf1_t = cload(bf1_d, [64, 1], dt.float32, "bf1")
        bf2_t = cload(bf2_d, [128, 1], dt.float32, "bf2")
        bf3_t = cload(bf3_d, [TC, NT], dt.float32, "bf3")
        bp1_t = cload(bp1_d, [128, 2], dt.float32, "bp1")
        bp2_t = cload(bp2_d, [128, 4], dt.float32, "bp2")
        bp3_t = cload(bp3_d, [TC, NT], dt.float32, "bp3")
        eps_t = const.tile([128, 1], dt.float32, name="eps")
        nc.vector.memset(eps_t[:], EPS)

        def block(b, e0):
            # ---- load + split ----
            x32 = big.tile([TC, NT, EW], dt.float32, tag="x32", bufs=2)
            nc.sync.dma_start(
                x32[:], x_d.ap()[b, :, e0:e0 + EW].rearrange(
                    "(c p) e -> p c e", p=TC))
            xh = big.tile([TC, NT, EW], dt.float16, tag="xh", bufs=2)
            nc.scalar.activation(xh[:], x32[:], AF.Identity, scale=SC)
            x8 = big.tile([TC, NT, 2, EW], dt.float8e4, tag="x8", bufs=2)
            nc.vector.scalar_tensor_tensor(
                x8[:, :, 0, :], x32[:], SC, xh[:], OP.mult, OP.subtract)
            nc.gpsimd.dma_start(x8[:, :, 1, :], x32[:])

            # ---- DFT: mains fp16 + corrections fp8 DR ----
            xr_t = big.tile([121, 3, EW], dt.float32, tag="xr", bufs=1)
            xi_t = big.tile([121, 3, EW], dt.float32, tag="xi", bufs=1)
            for mats, m8, dst in ((CH_t, C8_t, xr_t), (SH_t, S8_t, xi_t)):
                for ci, (f0, fw) in enumerate(FCH):
                    p = ps1.tile([128, 512], dt.float32, tag="psA", bufs=3)
                    for k in range(NT):
                        nc.tensor.matmul(p[0:fw, 0:EW], mats[:, k, f0:f0 + fw],
                                         xh[:, k, :], start=(k == 0), stop=False)
                    for k in range(NT):
                        nc.tensor.matmul(p[0:fw, 0:EW], m8[:, k, :, f0:f0 + fw],
                                         x8[:, k, :, :], start=False,
                                         stop=(k == NT - 1), perf_mode=DR)
                    nc.scalar.activation(dst[:, ci, :], p[0:121, 0:EW],
                                         AF.Identity, scale=1.0 / SC)

            # ---- mag2 (f32) ----
            sqr = tmp.tile([121, 3, EW], dt.float32, tag="sqr", bufs=1)
            nc.scalar.square(sqr[:], xr_t[:])
            sqi = tmp.tile([121, 3, EW], dt.float32, tag="sqi", bufs=1)
            nc.scalar.square(sqi[:], xi_t[:])
            mag2 = big.tile([121, 3, EW], dt.float32, tag="mag2", bufs=1)
            nc.vector.tensor_tensor(mag2[:], sqr[:], sqi[:], OP.add)

            # ---- transpose chunks into PSUM + top-20 threshold ----
            pthr = ps1.tile([128, 512], dt.float32, tag="psTH", bufs=1)
            m3s = []
            for ei, (ee0, ew) in enumerate(ECH):
                pt = ps1.tile([128, 512], dt.float32, tag="psA", bufs=3)
                for ci, (f0, fw) in enumerate(FCH):
                    nc.tensor.matmul(pt[0:ew, f0:f0 + fw],
                                     mag2[0:fw, ci, ee0:ee0 + ew],
                                     idf_t[0:fw, 0:fw], is_transpose=True,
                                     start=(ci == 0), stop=(ci == 2))
                m1 = tmp.tile([128, 8], dt.float32, tag=f"m1_{ei}")
                nc.vector.max(m1[0:ew, :], pt[0:ew, 0:FP])
                nc.vector.match_replace(pt[0:ew, 0:FP], m1[0:ew, :],
                                        pt[0:ew, 0:FP], -1e30)
                m2 = tmp.tile([128, 8], dt.float32, tag=f"m2_{ei}")
                nc.vector.max(m2[0:ew, :], pt[0:ew, 0:FP])
                nc.vector.match_replace(pt[0:ew, 0:FP], m2[0:ew, :],
                                        pt[0:ew, 0:FP], -1e30)
                m3 = tmp.tile([128, 8], dt.float32, tag=f"m3_{ei}")
                nc.vector.max(m3[0:ew, :], pt[0:ew, 0:FP])
                m3s.append(m3)
            for ei, (ee0, ew) in enumerate(ECH):
                nc.tensor.matmul(pthr[0:1, ee0:ee0 + ew], m3s[ei][0:ew, 3:4],
                                 idf_t[0:ew, 0:ew], is_transpose=True,
                                 start=(ei == 0), stop=(ei == 3))
            thr_row = tmp.tile([1, EW], dt.float32, tag="thr_row", bufs=2)
            nc.vector.tensor_copy(thr_row[:], pthr[0:1, 0:EW])
            ptb = ps1.tile([128, 512], dt.float32, tag="psTB", bufs=1)
            nc.tensor.matmul(ptb[:, 0:EW], ones_t[:], thr_row[:],
                             start=True, stop=True)

            # ---- mask + masked coefs (f-major) ----
            mask = big.tile([121, 3, EW], dt.float16, tag="mask", bufs=1)
            for ci in range(3):
                nc.vector.tensor_tensor(mask[:, ci, :], mag2[:, ci, :],
                                        ptb[0:121, 0:EW], OP.is_ge)
            xrm = big.tile([121, 3, EW], dt.float16, tag="xrm", bufs=2)
            nc.vector.tensor_tensor(xrm[:], xr_t[:], mask[:], OP.mult)
            xim = big.tile([121, 3, EW], dt.float16, tag="xim", bufs=2)
            nc.gpsimd.tensor_tensor(xim[:], xi_t[:], mask[:], OP.mult)

            # ---- iDFT -> nib (fp16), sq ----
            nib = big.tile([TC, NT, EW], dt.float16, tag="nib", bufs=2)
            for j in range(NT):
                t0 = TC * j
                p = ps1.tile([128, 512], dt.float32, tag="psA", bufs=3)
                for ci in range(3):
                    nc.tensor.matmul(p[0:TC, 0:EW], c2_t[:, ci, t0:t0 + TC],
                                     xrm[:, ci, :], start=(ci == 0), stop=False)
                    nc.tensor.matmul(p[0:TC, 0:EW], s2_t[:, ci, t0:t0 + TC],
                                     xim[:, ci, :], start=False, stop=(ci == 2))
                eng = nc.vector if j % 2 == 0 else nc.gpsimd
                eng.scalar_tensor_tensor(nib[:, j, :], p[0:TC, 0:EW], -1.0,
                                         x32[:, j, :], OP.mult, OP.add)
            sq = big.tile([TC, NT, EW], dt.float16, tag="sq", bufs=1)
            nc.vector.tensor_tensor(sq[:], nib[:], nib[:], OP.mult)

            # ---- MLP freq ----
            p = ps1.tile([128, 512], dt.float32, tag="psA", bufs=3)
            for ci in range(3):
                nc.tensor.matmul(p[0:64, 0:EW], w1c_t[:, ci, :], xrm[:, ci, :],
                                 start=(ci == 0), stop=False)
                nc.tensor.matmul(p[0:64, 0:EW], w1s_t[:, ci, :], xim[:, ci, :],
                                 start=False, stop=(ci == 2))
            h1f = tmp.tile([64, EW], dt.float8e4, tag="h1f", bufs=2)
            nc.scalar.activation(h1f[:], p[0:64, 0:EW], AF.Relu, bias=bf1_t[0:64, :])
            p = ps1.tile([128, 512], dt.float32, tag="psA", bufs=3)
            for tpair in range(3):
                nc.tensor.matmul(p[:, 0:EW], wf2x_t[:, tpair, :, :],
                                 x8[:, 2 * tpair:2 * tpair + 2, 1, :],
                                 start=(tpair == 0), stop=False, perf_mode=DR)
            nc.tensor.matmul(p[:, 0:EW], wf2h_t[:], h1f[:], start=False, stop=True)
            h2f = tmp.tile([128, EW], dt.float8e4, tag="h2f", bufs=2)
            nc.scalar.activation(h2f[:], p[:, 0:EW], AF.Relu, bias=bf2_t[:])

            # ---- band stats + norm ----
            mean16 = big.tile([TC, NT, EW], dt.float16, tag="mean16", bufs=1)
            std8 = big.tile([TC, NT, EW], dt.float8e4, tag="std8", bufs=2)
            orow = out_d.ap()[b, :, :].rearrange("t (s e) -> t s e", e=E)
            si = 0
            for j in range(NT):
                chunks = _band_slabs(j)
                pp = ps2.tile([128, 2, 512], dt.float32, tag="psBD")
                for k, cch in enumerate(chunks):
                    nc.tensor.matmul(pp[0:TC, 0, 0:EW], band_t[:, si + k, :],
                                     nib[:, cch, :], start=(k == 0),
                                     stop=(k == len(chunks) - 1))
                for k, cch in enumerate(chunks):
                    nc.tensor.matmul(pp[0:TC, 1, 0:EW], band_t[:, si + k, :],
                                     sq[:, cch, :], start=(k == 0),
                                     stop=(k == len(chunks) - 1))
                si += len(chunks)
                nc.scalar.copy(mean16[:, j, :], pp[0:TC, 0, 0:EW])
                msq = tmp.tile([TC, EW], dt.float16, tag="msq", bufs=2)
                nc.vector.tensor_tensor(msq[:], mean16[:, j, :],
                                        mean16[:, j, :], OP.mult)
                var16 = tmp.tile([TC, EW], dt.float16, tag="var16", bufs=2)
                nc.vector.tensor_tensor(var16[:], pp[0:TC, 1, 0:EW], msq[:],
                                        OP.subtract)
                std16 = tmp.tile([TC, EW], dt.float16, tag="std16", bufs=2)
                nc.scalar.activation(std16[:], var16[:], AF.Sqrt,
                                     bias=eps_t[0:TC, :])
                nc.scalar.activation(std8[:, j, :], var16[:], AF.Sqrt,
                                     bias=eps_t[0:TC, :])
                delta = tmp.tile([TC, EW], dt.float16, tag="delta", bufs=2)
                nc.vector.tensor_tensor(delta[:], nib[:, j, :], mean16[:, j, :],
                                        OP.subtract)
                norm = tmp.tile([TC, EW], dt.float32, tag="norm", bufs=3)
                nc.gpsimd.tensor_tensor(norm[:], delta[:], std16[:], OP.divide)
                nc.sync.dma_start(orow[TC * j:TC * (j + 1), 0, e0:e0 + EW],
                                  norm[:])
            mean8 = big.tile([TC, NT, EW], dt.float8e4, tag="mean8", bufs=2)
            nc.gpsimd.dma_start(mean8[:], mean16[:])

            # ---- MLP pred layers 1-2 (mean & std paths) ----
            h2ps = []
            for pi, src in enumerate((mean8, std8)):
                pq = ps2.tile([128, 2, 512], dt.float32, tag="psBD")
                for mi in range(2):
                    for tpair in range(3):
                        nc.tensor.matmul(
                            pq[:, mi, 0:EW],
                            wp1_t[:, tpair, :, 128 * mi:128 * (mi + 1)],
                            src[:, 2 * tpair:2 * tpair + 2, :],
                            start=(tpair == 0), stop=(tpair == 2), perf_mode=DR)
                h1p = tmp.tile([128, 2, EW], dt.float8e4, tag=f"h1p{pi}", bufs=2)
                for mi in range(2):
                    nc.scalar.activation(h1p[:, mi, :], pq[:, mi, 0:EW], AF.Relu,
                                         bias=bp1_t[:, mi:mi + 1])
                h2p = big.tile([128, 4, EW], dt.float8e4, tag=f"h2p{pi}", bufs=2)
                for half in range(2):
                    pr = ps2.tile([128, 2, 512], dt.float32, tag="psBD")
                    for mi2 in range(2):
                        mi = 2 * half + mi2
                        for tpair in range(3):
                            nc.tensor.matmul(
                                pr[:, mi2, 0:EW],
                                wp2x_t[:, tpair, :, 128 * mi:128 * (mi + 1)],
                                x8[:, 2 * tpair:2 * tpair + 2, 1, :],
                                start=(tpair == 0), stop=False, perf_mode=DR)
                        nc.tensor.matmul(pr[:, mi2, 0:EW],
                                         wp2h_t[:, :, 128 * mi:128 * (mi + 1)],
                                         h1p[:], start=False, stop=True,
                                         perf_mode=DR)
                        nc.scalar.activation(h2p[:, mi, :], pr[:, mi2, 0:EW],
                                             AF.Relu, bias=bp2_t[:, mi:mi + 1])
                h2ps.append(h2p)

            # ---- final layers, fused per-j, merged trio DMA ----
            for j in range(NT):
                pa = ps1.tile([128, 512], dt.float32, tag="psA", bufs=3)
                nc.tensor.matmul(pa[0:TC, 0:EW], wf3_t[:, j, :], h2f[:],
                                 start=True, stop=True)
                pb = ps1.tile([128, 512], dt.float32, tag="psA", bufs=3)
                for pr_ in range(2):
                    nc.tensor.matmul(pb[0:TC, 0:EW], wp3_t[:, pr_, :, j, :],
                                     h2ps[0][:, 2 * pr_:2 * pr_ + 2, :],
                                     start=(pr_ == 0), stop=(pr_ == 1),
                                     perf_mode=DR)
                pc = ps1.tile([128, 512], dt.float32, tag="psA", bufs=3)
                for pr_ in range(2):
                    nc.tensor.matmul(pc[0:TC, 0:EW], wp3_t[:, pr_, :, j, :],
                                     h2ps[1][:, 2 * pr_:2 * pr_ + 2, :],
                                     start=(pr_ == 0), stop=(pr_ == 1),
                                     perf_mode=DR)
                trio = tmp.tile([TC, 3, EW], dt.float32, tag="trio", bufs=2)
                nc.scalar.activation(trio[:, 0, :], pa[0:TC, 0:EW], AF.Identity,
                                     bias=bf3_t[:, j:j + 1], scale=1.0 / W8)
                nc.vector.tensor_scalar(trio[:, 1, :], pb[0:TC, 0:EW],
                                        bp3_t[:, j:j + 1], None, OP.add)
                nc.gpsimd.tensor_scalar(trio[:, 2, :], pc[0:TC, 0:EW],
                                        bp3_t[:, j:j + 1], None, OP.add)
                nc.sync.dma_start(orow[TC * j:TC * (j + 1), 1:4, e0:e0 + EW],
                                  trio[:])

        for b in range(BL):
            for (e0, _) in EH:
                block(b, e0)

    nc.compile()
    return nc


def _prep_inputs(inputs):
    c = _cache["consts"]
    w = _prep_weights(inputs)
    base = dict(
        CH=c["CH"], SH=c["SH"], C8=c["C8"], S8=c["S8"], c2=c["c2"], s2=c["s2"],
        band=c["band"], idf=c["idf"], ones=c["ones"], **w)
    x = np.ascontiguousarray(np.asarray(inputs["batch_x"], np.float32))
    in_maps = []
    for i in range(NCORES):
        m = dict(base)
        m["x"] = np.ascontiguousarray(x[i * BL:(i + 1) * BL])
        in_maps.append(m)
    return in_maps


def kernel(**inputs):
    from concourse.bass_utils import run_bass_kernel_spmd

    if "consts" not in _cache:
        _cache["consts"] = _host_constants()
    _cache["zero_bias"] = all(
        not np.any(np.asarray(inputs[k]))
        for k in ("bf1", "bf2", "bf3", "bp1", "bp2", "bp3"))
    if "nc" not in _cache:
        _cache["nc"] = _build_program()
    nc = _cache["nc"]
    in_maps = _prep_inputs(inputs)
    res = run_bass_kernel_spmd(nc, in_maps, core_ids=list(range(NCORES)))
    _cache["last_result"] = res
    out = np.concatenate([res.results[i]["out"] for i in range(NCORES)], axis=0)
    return out


# revision 5
# speedup vs baseline: 1.1144x; 1.0614x over previous
"""DualAN (normalization) Trainium2 Bass kernel — v2.

kernel(**inputs): FULL inputs (batch_x [32,720,862] f32 + MLP weights), pure
data parallel across 8 NeuronCores ([4,720,862] per core), FULL [32,720,3448]
f32 output.

Per (batch, 431-channel half) block, time-major [t, e] layouts:
  1. x split: xh_s = 1024*fp16(x) (ACT), xl8 = e4m3(1024*(x-xh)) (DVE),
     xh8 = e4m3(x) via gpsimd cast-DMA. All packed for fp8 DoubleRow.
  2. unfolded DFT (K=720): fp16 mains (CH @ xh_s, 1024-scaled psum) + fp8
     DoubleRow corrections ([e4m3(CH)|e4m3(1024 CL)] @ [xl8|xh8]) ->
     fp32-class Xr/Xi for exact top-20 ranking. Evac with scale 1/1024.
  3. mag2 = Xr^2 + Xi^2 (f32); PE-transpose into shared PSUM banks; top-20
     threshold per channel via 3x max8 + 2x in-place match_replace on PSUM.
  4. thr row via PE transposes + f32 outer-product broadcast; mask/masked
     coefs computed f-major (no mask transpose).
  5. iDFT (fp16) -> nib = x - x_filt (fp16); sq = nib^2.
  6. window mean/var via chunk-aligned 3-slab band matmuls (fp16, 1/24
     folded); norm = (nib - mean) * Rsqrt(var + eps).
  7. MLPs in fp8 DoubleRow (K=240/instr): freq-MLP layer 1 reads masked
     coefs through host-precomputed C2@Wf1 (x_filt never materialized for
     the MLP); pred-MLP shares nothing but weights between mean/std paths.
  8. outputs: norm DMA per j; pred trio merged [120,3,431] DMA per j.
"""

import numpy as np
from contextlib import ExitStack

B, S, E = 32, 720, 862
F = 361
FP = 363          # padded to 3*121
FP8 = 368         # fp8 DR weight slab stride (16B aligned)
PRED = 720
WIN = 24
EPS = 1e-5
NCORES = 8
BL = B // NCORES

TC = 120
NT = 6
EW = 431
FCH = [(0, 121), (121, 121), (242, 121)]
ECH = [(0, 128), (128, 128), (256, 128), (384, 47)]
EH = [(0, 431), (431, 431)]
SC = 1024.0       # hi/lo split scale
W8 = 64.0         # fp8 weight scale
M8 = 4.0          # fp8 mean/std scale

_cache = {}


def _f16(a):
    return np.asarray(a).astype(np.float16)


def _f8(a):
    import ml_dtypes
    return np.asarray(a, np.float32).astype(ml_dtypes.float8_e4m3)


def _band_slabs(j):
    """Chunks contributing to window rows of out-chunk j."""
    lo = max(j - 1, 0)
    hi = min(j + 1, NT - 1)
    return list(range(lo, hi + 1))


def _host_constants():
    t = np.arange(S, dtype=np.float64)
    f = np.arange(FP, dtype=np.float64)
    ang = 2.0 * np.pi * np.outer(t, f) / S          # [S, FP]
    C = np.cos(ang)
    Sn = -np.sin(ang)
    C[:, F:] = 0.0
    Sn[:, F:] = 0.0

    def pack_fwd(M):
        # [S, FP] f64 -> mains fp16 [TC, NT, FP], corr fp8 [TC, NT, 2, FP]
        Mh = M.astype(np.float32).astype(np.float16)          # hi
        Ml = (M.astype(np.float32) - Mh.astype(np.float32))   # lo
        mains = np.ascontiguousarray(
            Mh.reshape(NT, TC, FP).transpose(1, 0, 2))
        c8 = np.zeros((TC, NT, 2, FP8), np.float32)
        c8[:, :, 0, :FP] = Mh.astype(np.float32).reshape(NT, TC, FP).transpose(1, 0, 2)
        c8[:, :, 1, :FP] = (Ml * SC).reshape(NT, TC, FP).transpose(1, 0, 2)
        return mains, _f8(c8)

    CHm, C8 = pack_fwd(C)
    SHm, S8 = pack_fwd(Sn)

    # inverse DFT: x_filt[t] = sum_f c2[f,t] xr[f] + s2[f,t] xi[f]
    w = np.full(FP, 2.0)
    w[0] = 1.0
    w[360] = 1.0
    w[F:] = 0.0
    c2 = (w[:, None] * np.cos(ang.T) / S)           # [FP, S]
    s2 = (w[:, None] * (-np.sin(ang.T)) / S)
    c2[F:] = 0.0
    s2[F:] = 0.0
    c2_t = _f16(-c2.reshape(3, 121, S).transpose(1, 0, 2))  # negated: psum = -x_filt
    s2_t = _f16(-s2.reshape(3, 121, S).transpose(1, 0, 2))

    # band slab matrices (1/24 folded): [TC(src), 16, TC(out)]
    slab_list = []   # (j, chunk) in emission order
    for j in range(NT):
        for c in _band_slabs(j):
            slab_list.append((j, c))
    band = np.zeros((TC, len(slab_list), TC), np.float64)
    for si, (j, c) in enumerate(slab_list):
        for tt in range(TC):
            g = TC * j + tt
            for q in range(g - WIN // 2, g + WIN // 2):
                qq = min(max(q, 0), S - 1)
                if qq // TC == c:
                    band[qq % TC, si, tt] += 1.0 / WIN
    ident = np.eye(128, dtype=np.float32)
    return dict(
        CH=CHm, SH=SHm, C8=C8, S8=S8, c2=c2_t, s2=s2_t,
        band=_f16(band), slab_list=slab_list, idf=ident,
        ones=np.ones((1, 128), np.float32),
        idh=(np.eye(128) / 1024.0).astype(np.float16),
    )


def _prep_weights(inputs):
    """Host-side packing of MLP weights into fp16/fp8 DoubleRow layouts."""
    import ml_dtypes  # noqa: F401
    c = _cache["consts"]
    Wf1 = np.asarray(inputs["Wf1"], np.float32)     # [720, 64]
    Wf2 = np.asarray(inputs["Wf2"], np.float32)     # [784, 128]
    Wf3 = np.asarray(inputs["Wf3"], np.float32)     # [128, 720]
    Wp1 = np.asarray(inputs["Wp1"], np.float32)     # [720, 256]
    Wp2 = np.asarray(inputs["Wp2"], np.float32)     # [976, 512]
    Wp3 = np.asarray(inputs["Wp3"], np.float32)     # [512, 720]

    # W1C/W1S: [FP, 64] = c2 @ Wf1 (fp16 lhsT [121, 3, 64])
    t = np.arange(S, dtype=np.float64)
    f = np.arange(FP, dtype=np.float64)
    ang = 2.0 * np.pi * np.outer(f, t) / S          # [FP, S]
    w = np.full(FP, 2.0); w[0] = 1.0; w[360] = 1.0; w[F:] = 0.0
    c2 = w[:, None] * np.cos(ang) / S
    s2 = w[:, None] * (-np.sin(ang)) / S
    c2[F:] = 0.0; s2[F:] = 0.0
    W1C = (c2 @ Wf1.astype(np.float64)).astype(np.float32)   # [FP, 64]
    W1S = (s2 @ Wf1.astype(np.float64)).astype(np.float32)

    def dr_pack_k(Wk, m):
        # [720, m] -> [TC, 3, 2, m] pairing k-chunks (2t, 2t+1)
        return _f8(W8 * Wk.reshape(3, 2, TC, m).transpose(2, 0, 1, 3))

    d = dict(
        w1c=_f16(W1C.reshape(3, 121, 64).transpose(1, 0, 2)),
        w1s=_f16(W1S.reshape(3, 121, 64).transpose(1, 0, 2)),
        wf2x=dr_pack_k(Wf2[64:], 128),
        wf2h=_f8(W8 * Wf2[:64]),                     # [64, 128]
        wf3=_f8(W8 * Wf3.reshape(2, 64, NT, TC).transpose(1, 0, 2, 3)),
        wp1=dr_pack_k(Wp1, 256),
        wp2x=dr_pack_k(Wp2[256:], 512),
        wp2h=_f8(W8 * Wp2[:256].reshape(2, 128, 512).transpose(1, 0, 2)),
        wp3=_f8(W8 * Wp3.reshape(2, 2, 128, NT, TC).transpose(2, 0, 1, 3, 4)),
        # wp3: [128, pair, slab, 6, 120]: slab s of pair p = kc (2p+s)
        bf1=np.asarray(inputs["bf1"], np.float32).reshape(64, 1),
        bf2=np.asarray(inputs["bf2"], np.float32).reshape(128, 1),
        bf3=np.asarray(inputs["bf3"], np.float32).reshape(NT, TC).T.copy(),
        bp1=np.asarray(inputs["bp1"], np.float32).reshape(2, 128).T.copy(),
        bp2=np.asarray(inputs["bp2"], np.float32).reshape(4, 128).T.copy(),
        bp3=np.asarray(inputs["bp3"], np.float32).reshape(NT, TC).T.copy(),
    )
    return d


def _build_program():
    import concourse.tile as tile
    from concourse import bacc, mybir

    dt = mybir.dt
    AF = mybir.ActivationFunctionType
    OP = mybir.AluOpType
    DR = mybir.MatmulPerfMode.DoubleRow
    ZB = _cache.get("zero_bias", False)
    c = _cache["consts"]
    slab_list = c["slab_list"]

    nc = bacc.Bacc("TRN2", target_bir_lowering=False, debug=False)

    x_d = nc.dram_tensor("x", [BL, S, E], dt.float32, kind="ExternalInput")
    CH_d = nc.dram_tensor("CH", [TC, NT, FP], dt.float16, kind="ExternalInput")
    SH_d = nc.dram_tensor("SH", [TC, NT, FP], dt.float16, kind="ExternalInput")
    C8_d = nc.dram_tensor("C8", [TC, NT, 2, FP8], dt.float8e4, kind="ExternalInput")
    S8_d = nc.dram_tensor("S8", [TC, NT, 2, FP8], dt.float8e4, kind="ExternalInput")
    c2_d = nc.dram_tensor("c2", [121, 3, S], dt.float16, kind="ExternalInput")
    s2_d = nc.dram_tensor("s2", [121, 3, S], dt.float16, kind="ExternalInput")
    band_d = nc.dram_tensor("band", [TC, len(slab_list), TC], dt.float16,
                            kind="ExternalInput")
    idf_d = nc.dram_tensor("idf", [128, 128], dt.float32, kind="ExternalInput")
    idh_d = nc.dram_tensor("idh", [128, 128], dt.float16, kind="ExternalInput")
    ones_d = nc.dram_tensor("ones", [1, 128], dt.float32, kind="ExternalInput")
    w1c_d = nc.dram_tensor("w1c", [121, 3, 64], dt.float16, kind="ExternalInput")
    w1s_d = nc.dram_tensor("w1s", [121, 3, 64], dt.float16, kind="ExternalInput")
    wf2x_d = nc.dram_tensor("wf2x", [TC, 3, 2, 128], dt.float8e4, kind="ExternalInput")
    wf2h_d = nc.dram_tensor("wf2h", [64, 128], dt.float8e4, kind="ExternalInput")
    wf3_d = nc.dram_tensor("wf3", [64, 2, NT, TC], dt.float8e4, kind="ExternalInput")
    wp1_d = nc.dram_tensor("wp1", [TC, 3, 2, 256], dt.float8e4, kind="ExternalInput")
    wp2x_d = nc.dram_tensor("wp2x", [TC, 3, 2, 512], dt.float8e4, kind="ExternalInput")
    wp2h_d = nc.dram_tensor("wp2h", [128, 2, 512], dt.float8e4, kind="ExternalInput")
    wp3_d = nc.dram_tensor("wp3", [128, 2, 2, NT, TC], dt.float8e4, kind="ExternalInput")
    bf1_d = nc.dram_tensor("bf1", [64, 1], dt.float32, kind="ExternalInput")
    bf2_d = nc.dram_tensor("bf2", [128, 1], dt.float32, kind="ExternalInput")
    bf3_d = nc.dram_tensor("bf3", [TC, NT], dt.float32, kind="ExternalInput")
    bp1_d = nc.dram_tensor("bp1", [128, 2], dt.float32, kind="ExternalInput")
    bp2_d = nc.dram_tensor("bp2", [128, 4], dt.float32, kind="ExternalInput")
    bp3_d = nc.dram_tensor("bp3", [TC, NT], dt.float32, kind="ExternalInput")
    out_d = nc.dram_tensor("out", [BL, S, 4 * E], dt.float32, kind="ExternalOutput")

    with tile.TileContext(nc) as tc_, ExitStack() as ctx:
        const = ctx.enter_context(tc_.tile_pool(name="const", bufs=1))
        big = ctx.enter_context(tc_.tile_pool(name="big", bufs=1))
        tmp = ctx.enter_context(tc_.tile_pool(name="tmp", bufs=1))
        ps1 = ctx.enter_context(tc_.tile_pool(name="ps1", bufs=1, space="PSUM"))
        ps2 = ctx.enter_context(tc_.tile_pool(name="ps2", bufs=1, space="PSUM"))

        def cload(d, shape, dtype, name):
            t_ = const.tile(shape, dtype, name=name)
            nc.sync.dma_start(t_[:], d.ap()[:])
            return t_

        CH_t = cload(CH_d, [TC, NT, FP], dt.float16, "CH")
        SH_t = cload(SH_d, [TC, NT, FP], dt.float16, "SH")
        C8_t = cload(C8_d, [TC, NT, 2, FP8], dt.float8e4, "C8")
        S8_t = cload(S8_d, [TC, NT, 2, FP8], dt.float8e4, "S8")
        c2_t = cload(c2_d, [121, 3, S], dt.float16, "c2")
        s2_t = cload(s2_d, [121, 3, S], dt.float16, "s2")
        band_t = cload(band_d, [TC, len(slab_list), TC], dt.float16, "band")
        idf_t = cload(idf_d, [128, 128], dt.float32, "idf")
        idh_t = cload(idh_d, [128, 128], dt.float16, "idh")
        ones_t = cload(ones_d, [1, 128], dt.float32, "ones")
        w1c_t = cload(w1c_d, [121, 3, 64], dt.float16, "w1c")
        w1s_t = cload(w1s_d, [121, 3, 64], dt.float16, "w1s")
        wf2x_t = cload(wf2x_d, [TC, 3, 2, 128], dt.float8e4, "wf2x")
        wf2h_t = cload(wf2h_d, [64, 128], dt.float8e4, "wf2h")
        wf3_t = cload(wf3_d, [64, 2, NT, TC], dt.float8e4, "wf3")
        wp1_t = cload(wp1_d, [TC, 3, 2, 256], dt.float8e4, "wp1")
        wp2x_t = cload(wp2x_d, [TC, 3, 2, 512], dt.float8e4, "wp2x")
        wp2h_t = cload(wp2h_d, [128, 2, 512], dt.float8e4, "wp2h")
        wp3_t = cload(wp3_d, [128, 2, 2, NT, TC], dt.float8e4, "wp3")
        bf1_t = cload(bf1_d, [64, 1], dt.float32, "bf1")
        bf2_t = cload(bf2_d, [128, 1], dt.float32, "bf2")
        bf3_t = cload(bf3_d, [TC, NT], dt.float32, "bf3")
        bp1_t = cload(bp1_d, [128, 2], dt.float32, "bp1")
        bp2_t = cload(bp2_d, [128, 4], dt.float32, "bp2")
        bp3_t = cload(bp3_d, [TC, NT], dt.float32, "bp3")
        eps_t = const.tile([128, 1], dt.float32, name="eps")
        nc.vector.memset(eps_t[:], EPS)

        def block(b, e0):
            # ---- load + split ----
            x32 = big.tile([TC, NT, EW], dt.float32, tag="x32", bufs=2)
            nc.sync.dma_start(
                x32[:], x_d.ap()[b, :, e0:e0 + EW].rearrange(
                    "(c p) e -> p c e", p=TC))
            xh = big.tile([TC, NT, EW], dt.float16, tag="xh", bufs=2)
            nc.scalar.activation(xh[:], x32[:], AF.Identity, scale=SC)
            x8 = big.tile([TC, NT, 2, EW], dt.float8e4, tag="x8", bufs=2)
            nc.vector.scalar_tensor_tensor(
                x8[:, :, 0, :], x32[:], SC, xh[:], OP.mult, OP.subtract)
            nc.gpsimd.dma_start(x8[:, :, 1, :], x32[:])

            # ---- DFT: mains fp16 + corrections fp8 DR ----
            xr_t = big.tile([121, 3, EW], dt.float32, tag="xr", bufs=1)
            xi_t = big.tile([121, 3, EW], dt.float32, tag="xi", bufs=1)
            for mats, m8, dst in ((CH_t, C8_t, xr_t), (SH_t, S8_t, xi_t)):
                for ci, (f0, fw) in enumerate(FCH):
                    p = ps1.tile([128, 512], dt.float32, tag="psA", bufs=3)
                    for k in range(NT):
                        nc.tensor.matmul(p[0:fw, 0:EW], mats[:, k, f0:f0 + fw],
                                         xh[:, k, :], start=(k == 0), stop=False)
                    for k in range(NT):
                        nc.tensor.matmul(p[0:fw, 0:EW], m8[:, k, :, f0:f0 + fw],
                                         x8[:, k, :, :], start=False,
                                         stop=(k == NT - 1), perf_mode=DR)
                    nc.scalar.activation(dst[:, ci, :], p[0:121, 0:EW],
                                         AF.Identity, scale=1.0 / SC)

            # ---- mag2 (f32) ----
            sqr = tmp.tile([121, 3, EW], dt.float32, tag="sqr", bufs=1)
            nc.scalar.square(sqr[:], xr_t[:])
            sqi = tmp.tile([121, 3, EW], dt.float32, tag="sqi", bufs=1)
            nc.scalar.square(sqi[:], xi_t[:])
            mag2 = big.tile([121, 3, EW], dt.float32, tag="mag2", bufs=1)
            nc.vector.tensor_tensor(mag2[:], sqr[:], sqi[:], OP.add)

            # ---- transpose chunks into PSUM + top-20 threshold ----
            pthr = ps1.tile([128, 512], dt.float32, tag="psTH", bufs=1)
            m3s = []
            for ei, (ee0, ew) in enumerate(ECH):
                pt = ps1.tile([128, 512], dt.float32, tag="psA", bufs=3)
                for ci, (f0, fw) in enumerate(FCH):
                    nc.tensor.matmul(pt[0:ew, f0:f0 + fw],
                                     mag2[0:fw, ci, ee0:ee0 + ew],
                                     idf_t[0:fw, 0:fw], is_transpose=True,
                                     start=(ci == 0), stop=(ci == 2))
                m1 = tmp.tile([128, 8], dt.float32, tag=f"m1_{ei}")
                nc.vector.max(m1[0:ew, :], pt[0:ew, 0:FP])
                nc.vector.match_replace(pt[0:ew, 0:FP], m1[0:ew, :],
                                        pt[0:ew, 0:FP], -1e30)
                m2 = tmp.tile([128, 8], dt.float32, tag=f"m2_{ei}")
                nc.vector.max(m2[0:ew, :], pt[0:ew, 0:FP])
                nc.vector.match_replace(pt[0:ew, 0:FP], m2[0:ew, :],
                                        pt[0:ew, 0:FP], -1e30)
                m3 = tmp.tile([128, 8], dt.float32, tag=f"m3_{ei}")
                nc.vector.max(m3[0:ew, :], pt[0:ew, 0:FP])
                m3s.append(m3)
            for ei, (ee0, ew) in enumerate(ECH):
                nc.tensor.matmul(pthr[0:1, ee0:ee0 + ew], m3s[ei][0:ew, 3:4],
                                 idf_t[0:ew, 0:ew], is_transpose=True,
                                 start=(ei == 0), stop=(ei == 3))
            thr_row = tmp.tile([1, EW], dt.float32, tag="thr_row", bufs=2)
            nc.vector.tensor_copy(thr_row[:], pthr[0:1, 0:EW])
            ptb = ps1.tile([128, 512], dt.float32, tag="psTB", bufs=1)
            nc.tensor.matmul(ptb[:, 0:EW], ones_t[:], thr_row[:],
                             start=True, stop=True)

            # ---- mask + masked coefs (f-major) ----
            mask = big.tile([121, 3, EW], dt.float16, tag="mask", bufs=1)
            for ci in range(3):
                nc.vector.tensor_tensor(mask[:, ci, :], mag2[:, ci, :],
                                        ptb[0:121, 0:EW], OP.is_ge)
            xrm = big.tile([121, 3, EW], dt.float16, tag="xrm", bufs=2)
            nc.vector.tensor_tensor(xrm[:], xr_t[:], mask[:], OP.mult)
            xim = big.tile([121, 3, EW], dt.float16, tag="xim", bufs=2)
            nc.gpsimd.tensor_tensor(xim[:], xi_t[:], mask[:], OP.mult)

            # ---- iDFT -> nib (fp16), sq ----
            nib = big.tile([TC, NT, EW], dt.float16, tag="nib", bufs=2)
            for j in range(NT):
                t0 = TC * j
                p = ps1.tile([128, 512], dt.float32, tag="psA", bufs=3)
                for ci in range(3):
                    nc.tensor.matmul(p[0:TC, 0:EW], c2_t[:, ci, t0:t0 + TC],
                                     xrm[:, ci, :], start=(ci == 0), stop=False)
                    nc.tensor.matmul(p[0:TC, 0:EW], s2_t[:, ci, t0:t0 + TC],
                                     xim[:, ci, :], start=False, stop=(ci == 2))
                eng = nc.vector if j % 2 == 0 else nc.gpsimd
                eng.scalar_tensor_tensor(nib[:, j, :], p[0:TC, 0:EW], -1.0,
                                         x32[:, j, :], OP.mult, OP.add)
            sq = big.tile([TC, NT, EW], dt.float16, tag="sq", bufs=1)
            nc.vector.tensor_tensor(sq[:], nib[:], nib[:], OP.mult)

            # ---- MLP freq ----
            p = ps1.tile([128, 512], dt.float32, tag="psA", bufs=3)
            for ci in range(3):
                nc.tensor.matmul(p[0:64, 0:EW], w1c_t[:, ci, :], xrm[:, ci, :],
                                 start=(ci == 0), stop=False)
                nc.tensor.matmul(p[0:64, 0:EW], w1s_t[:, ci, :], xim[:, ci, :],
                                 start=False, stop=(ci == 2))
            h1f = tmp.tile([64, EW], dt.float8e4, tag="h1f", bufs=2)
            nc.scalar.activation(h1f[:], p[0:64, 0:EW], AF.Relu, bias=bf1_t[0:64, :])
            p = ps1.tile([128, 512], dt.float32, tag="psA", bufs=3)
            for tpair in range(3):
                nc.tensor.matmul(p[:, 0:EW], wf2x_t[:, tpair, :, :],
                                 x8[:, 2 * tpair:2 * tpair + 2, 1, :],
                                 start=(tpair == 0), stop=False, perf_mode=DR)
            nc.tensor.matmul(p[:, 0:EW], wf2h_t[:], h1f[:], start=False, stop=True)
            h2f = tmp.tile([128, EW], dt.float8e4, tag="h2f", bufs=2)
            nc.scalar.activation(h2f[:], p[:, 0:EW], AF.Relu, bias=bf2_t[:])

            # ---- band stats + norm ----
            mean16 = big.tile([TC, NT, EW], dt.float16, tag="mean16", bufs=1)
            std8 = big.tile([TC, NT, EW], dt.float8e4, tag="std8", bufs=2)
            orow = out_d.ap()[b, :, :].rearrange("t (s e) -> t s e", e=E)
            si = 0
            for j in range(NT):
                chunks = _band_slabs(j)
                pp = ps2.tile([128, 2, 512], dt.float32, tag="psBD")
                for k, cch in enumerate(chunks):
                    nc.tensor.matmul(pp[0:TC, 0, 0:EW], band_t[:, si + k, :],
                                     nib[:, cch, :], start=(k == 0),
                                     stop=(k == len(chunks) - 1))
                for k, cch in enumerate(chunks):
                    nc.tensor.matmul(pp[0:TC, 1, 0:EW], band_t[:, si + k, :],
                                     sq[:, cch, :], start=(k == 0),
                                     stop=(k == len(chunks) - 1))
                si += len(chunks)
                nc.scalar.copy(mean16[:, j, :], pp[0:TC, 0, 0:EW])
                msq = tmp.tile([TC, EW], dt.float16, tag="msq", bufs=2)
                nc.vector.tensor_tensor(msq[:], mean16[:, j, :],
                                        mean16[:, j, :], OP.mult)
                var16 = tmp.tile([TC, EW], dt.float16, tag="var16", bufs=2)
                nc.vector.tensor_tensor(var16[:], pp[0:TC, 1, 0:EW], msq[:],
                                        OP.subtract)
                std16 = tmp.tile([TC, EW], dt.float16, tag="std16", bufs=2)
                nc.scalar.activation(std16[:], var16[:], AF.Sqrt,
                                     bias=eps_t[0:TC, :])
                nc.scalar.activation(std8[:, j, :], var16[:], AF.Sqrt,
                                     bias=eps_t[0:TC, :])
                delta = tmp.tile([TC, EW], dt.float16, tag="delta", bufs=2)
                nc.vector.tensor_tensor(delta[:], nib[:, j, :], mean16[:, j, :],
                                        OP.subtract)
                norm = tmp.tile([TC, EW], dt.float32, tag="norm", bufs=3)
                nc.gpsimd.tensor_tensor(norm[:], delta[:], std16[:], OP.divide)
                nc.sync.dma_start(orow[TC * j:TC * (j + 1), 0, e0:e0 + EW],
                                  norm[:])
            mean8 = big.tile([TC, NT, EW], dt.float8e4, tag="mean8", bufs=2)
            nc.gpsimd.dma_start(mean8[:], mean16[:])

            # ---- MLP pred layers 1-2 (mean & std paths) ----
            h2ps = []
            for pi, src in enumerate((mean8, std8)):
                pq = ps2.tile([128, 2, 512], dt.float32, tag="psBD")
                for mi in range(2):
                    for tpair in range(3):
                        nc.tensor.matmul(
                            pq[:, mi, 0:EW],
                            wp1_t[:, tpair, :, 128 * mi:128 * (mi + 1)],
                            src[:, 2 * tpair:2 * tpair + 2, :],
                            start=(tpair == 0), stop=(tpair == 2), perf_mode=DR)
                h1p = tmp.tile([128, 2, EW], dt.float8e4, tag=f"h1p{pi}", bufs=2)
                for mi in range(2):
                    nc.scalar.activation(h1p[:, mi, :], pq[:, mi, 0:EW], AF.Relu,
                                         bias=bp1_t[:, mi:mi + 1])
                h2p = big.tile([128, 4, EW], dt.float8e4, tag=f"h2p{pi}", bufs=2)
                for half in range(2):
                    pr = ps2.tile([128, 2, 512], dt.float32, tag="psBD")
                    for mi2 in range(2):
                        mi = 2 * half + mi2
                        for tpair in range(3):
                            nc.tensor.matmul(
                                pr[:, mi2, 0:EW],
                                wp2x_t[:, tpair, :, 128 * mi:128 * (mi + 1)],
                                x8[:, 2 * tpair:2 * tpair + 2, 1, :],
                                start=(tpair == 0), stop=False, perf_mode=DR)
                        nc.tensor.matmul(pr[:, mi2, 0:EW],
                                         wp2h_t[:, :, 128 * mi:128 * (mi + 1)],
                                         h1p[:], start=False, stop=True,
                                         perf_mode=DR)
                        nc.scalar.activation(h2p[:, mi, :], pr[:, mi2, 0:EW],
                                             AF.Relu, bias=bp2_t[:, mi:mi + 1])
                h2ps.append(h2p)

            # ---- final layers, fused per-j, merged trio DMA ----
            for j in range(NT):
                pa = ps1.tile([128, 512], dt.float32, tag="psA", bufs=3)
                nc.tensor.matmul(pa[0:TC, 0:EW], wf3_t[:, j, :], h2f[:],
                                 start=True, stop=True)
                pb = ps1.tile([128, 512], dt.float32, tag="psA", bufs=3)
                for pr_ in range(2):
                    nc.tensor.matmul(pb[0:TC, 0:EW], wp3_t[:, pr_, :, j, :],
                                     h2ps[0][:, 2 * pr_:2 * pr_ + 2, :],
                                     start=(pr_ == 0), stop=(pr_ == 1),
                                     perf_mode=DR)
                pc = ps1.tile([128, 512], dt.float32, tag="psA", bufs=3)
                for pr_ in range(2):
                    nc.tensor.matmul(pc[0:TC, 0:EW], wp3_t[:, pr_, :, j, :],
                                     h2ps[1][:, 2 * pr_:2 * pr_ + 2, :],
                                     start=(pr_ == 0), stop=(pr_ == 1),
                                     perf_mode=DR)
                trio = tmp.tile([TC, 3, EW], dt.float32, tag="trio", bufs=2)
                nc.scalar.activation(trio[:, 0, :], pa[0:TC, 0:EW], AF.Identity,
                                     bias=bf3_t[:, j:j + 1], scale=1.0 / W8)
                nc.vector.tensor_scalar(trio[:, 1, :], pb[0:TC, 0:EW],
                                        bp3_t[:, j:j + 1], None, OP.add)
                nc.gpsimd.tensor_scalar(trio[:, 2, :], pc[0:TC, 0:EW],
                                        bp3_t[:, j:j + 1], None, OP.add)
                nc.sync.dma_start(orow[TC * j:TC * (j + 1), 1:4, e0:e0 + EW],
                                  trio[:])

        for b in range(BL):
            for (e0, _) in EH:
                block(b, e0)

    nc.compile()
    return nc


def _prep_inputs(inputs):
    c = _cache["consts"]
    w = _prep_weights(inputs)
    base = dict(
        CH=c["CH"], SH=c["SH"], C8=c["C8"], S8=c["S8"], c2=c["c2"], s2=c["s2"],
        band=c["band"], idf=c["idf"], idh=c["idh"], ones=c["ones"], **w)
    x = np.ascontiguousarray(np.asarray(inputs["batch_x"], np.float32))
    in_maps = []
    for i in range(NCORES):
        m = dict(base)
        m["x"] = np.ascontiguousarray(x[i * BL:(i + 1) * BL])
        in_maps.append(m)
    return in_maps


def kernel(**inputs):
    from concourse.bass_utils import run_bass_kernel_spmd

    if "consts" not in _cache:
        _cache["consts"] = _host_constants()
    _cache["zero_bias"] = all(
        not np.any(np.asarray(inputs[k]))
        for k in ("bf1", "bf2", "bf3", "bp1", "bp2", "bp3"))
    if "nc" not in _cache:
        _cache["nc"] = _build_program()
    nc = _cache["nc"]
    in_maps = _prep_inputs(inputs)
    res = run_bass_kernel_spmd(nc, in_maps, core_ids=list(range(NCORES)))
    _cache["last_result"] = res
    out = np.concatenate([res.results[i]["out"] for i in range(NCORES)], axis=0)
    return out
